# revision 1
# baseline (speedup 1.0000x reference)
"""Trainium2 Bass kernel for a 3-layer GRU (B=128, T=512, IN=128, H=1024, PRED=96).

Strategy: data-parallel over batch across 8 NeuronCores (B_core=16).
Per core, per layer:
  phase A (inproj): xi = W_ih @ h_prev_seq as batched N=512 GEMMs (bf16),
      biases (b_ih, + b_hh for r/z gates) folded into the PSUM->SBUF
      evacuation on the scalar engine; xi round-trips through DRAM.
  phase B (recurrence): 512 sequential steps; per step 192 self-loading
      bf16 matmuls (K=128, M=128, N=16) accumulate W_hh @ h_t into one
      PSUM bank laid out gate-major [128, 24 chunks x 16 batch]; gate
      math on DVE/ACT in the same layout; hidden state kept fp32 with a
      bf16 shadow for the tensor engine.
Final FC done on-chip; host only transposes [96,16] -> [16,96] per core.
"""

import numpy as np
from contextlib import ExitStack

import concourse.bass as bass
import concourse.bacc as bacc
import concourse.mybir as mybir
import concourse.tile as tile
from concourse.bass_utils import run_bass_kernel_spmd

try:
    from ml_dtypes import bfloat16 as np_bf16
except ImportError:  # pragma: no cover
    import jax.numpy as jnp

    np_bf16 = jnp.bfloat16

B, T, IN, H, NLAYERS, PRED = 128, 512, 128, 1024, 3, 96
NCORES = 8
BC = B // NCORES  # 16
G = 3 * H  # 3072
NK = H // 128  # 8
NM = G // 128  # 24
BLK = 16  # recurrence steps per For_i iteration
BLKA = 4  # early sub-block of xi steps
F32, BF16 = mybir.dt.float32, mybir.dt.bfloat16
AF = mybir.ActivationFunctionType


def build(T_=T):
    nt = T_ * BC
    nblk = T_ // BLK
    ntile = nt // 512 if nt >= 512 else 0
    nc = bacc.Bacc("TRN2", target_bir_lowering=False, debug=False,
                   num_devices=NCORES)

    xT = nc.dram_tensor("xT", [128, nt], BF16, kind="ExternalInput")
    wihs = [
        nc.dram_tensor("wih0", [128, 1, G], BF16, kind="ExternalInput"),
        nc.dram_tensor("wih1", [128, NK, G], BF16, kind="ExternalInput"),
        nc.dram_tensor("wih2", [128, NK, G], BF16, kind="ExternalInput"),
    ]
    whhs = [nc.dram_tensor(f"whh{l}", [128, NK, G], BF16, kind="ExternalInput")
            for l in range(NLAYERS)]
    bizs = [nc.dram_tensor(f"biz{l}", [128, NM], F32, kind="ExternalInput")
            for l in range(NLAYERS)]
    bhns = [nc.dram_tensor(f"bhn{l}", [128, NK * BC], F32, kind="ExternalInput")
            for l in range(NLAYERS)]
    fcw = nc.dram_tensor("fcw", [128, NK, PRED], BF16, kind="ExternalInput")
    fcb = nc.dram_tensor("fcb", [PRED, 1], F32, kind="ExternalInput")
    out = nc.dram_tensor("out", [PRED, BC], F32, kind="ExternalOutput")

    with tile.TileContext(nc) as tc, ExitStack() as ctx:
        wpool = ctx.enter_context(tc.tile_pool(name="w", bufs=1))
        cpool = ctx.enter_context(tc.tile_pool(name="const", bufs=1))
        xpool = ctx.enter_context(tc.tile_pool(name="xT", bufs=1))
        rpool = ctx.enter_context(tc.tile_pool(name="rhs", bufs=2))
        epool = ctx.enter_context(tc.tile_pool(name="ev", bufs=3))
        xipool = ctx.enter_context(tc.tile_pool(name="xib", bufs=2))
        wnpool = ctx.enter_context(tc.tile_pool(name="win", bufs=2))
        tpool = ctx.enter_context(tc.tile_pool(name="tmp", bufs=2))
        spool = ctx.enter_context(tc.tile_pool(name="state", bufs=1))
        pspool = ctx.enter_context(tc.tile_pool(name="ps", bufs=3, space="PSUM"))
        fcpool = ctx.enter_context(tc.tile_pool(name="psfc", bufs=1, space="PSUM"))
        pgpool = ctx.enter_context(tc.tile_pool(name="pg", bufs=3, space="PSUM"))
        dpool = ctx.enter_context(tc.tile_pool(name="dram", bufs=1, space="DRAM"))
        hqpool = ctx.enter_context(tc.tile_pool(name="hseq", bufs=2, space="DRAM"))

        # persistent state
        h32 = spool.tile([128, NK, BC], F32, tag="h32")
        h16 = spool.tile([128, NK, BC], BF16, tag="h16")

        xT_sb = xpool.tile([128, nt], BF16, tag="xT")
        nc.sync.dma_start(xT_sb[:], xT[:])
        fcw_sb = cpool.tile([128, NK, PRED], BF16, tag="fcw")
        nc.sync.dma_start(fcw_sb[:], fcw[:])
        fcb_sb = cpool.tile([PRED, 1], F32, tag="fcb")
        nc.sync.dma_start(fcb_sb[:], fcb[:])

        hseq_prev = None
        for l in range(NLAYERS):
            nkl = 1 if l == 0 else NK
            wih_sb = wpool.tile([128, NK, G], BF16, tag="wih")
            nc.sync.dma_start(wih_sb[:, 0:nkl, :], wihs[l][:])
            whh_sb = wpool.tile([128, NK, G], BF16, tag="whh")
            nc.sync.dma_start(whh_sb[:], whhs[l][:])
            biz_sb = cpool.tile([128, NM], F32, tag="biz")
            nc.sync.dma_start(biz_sb[:], bizs[l][:])
            bhn_sb = cpool.tile([128, NK * BC], F32, tag="bhn")
            nc.sync.dma_start(bhn_sb[:], bhns[l][:])

            xi = dpool.tile([128, NM, nt], BF16, tag="xi")

            # ---------------- phase A: input projection ----------------
            def inproj_body(n):
                noff = n * 512
                if l == 0:
                    rhs_of = [xT_sb[:, bass.ds(noff, 512)]]
                else:
                    rhsb = rpool.tile([128, NK, 512], BF16, tag="rhs")
                    nc.sync.dma_start(rhsb[:], hseq_prev[:, :, bass.ds(noff, 512)])
                    rhs_of = [rhsb[:, k, :] for k in range(NK)]
                for m in range(NM):
                    ps = pspool.tile([128, 512], F32, tag="ps")
                    for k in range(nkl):
                        nc.tensor.matmul(ps[:], wih_sb[:, k, m * 128:(m + 1) * 128],
                                         rhs_of[k], start=(k == 0), stop=(k == nkl - 1))
                    ev = epool.tile([128, 512], BF16, tag="ev")
                    nc.scalar.activation(ev[:], ps[:], AF.Identity, bias=biz_sb[:, m:m + 1])
                    nc.sync.dma_start(xi[:, m, bass.ds(noff, 512)], ev[:])

            if ntile:
                with tc.For_i(0, ntile) as n:
                    inproj_body(n)
            else:  # tiny-T debug path
                for m in range(NM):
                    ps = pspool.tile([128, nt], F32, tag="ps")
                    if l == 0:
                        for k in range(1):
                            nc.tensor.matmul(ps[:], wih_sb[:, k, m * 128:(m + 1) * 128],
                                             xT_sb[:], start=True, stop=True)
                    else:
                        rhsb = rpool.tile([128, NK, nt], BF16, tag="rhs")
                        nc.sync.dma_start(rhsb[:], hseq_prev[:])
                        for k in range(NK):
                            nc.tensor.matmul(ps[:], wih_sb[:, k, m * 128:(m + 1) * 128],
                                             rhsb[:, k, :], start=(k == 0), stop=(k == NK - 1))
                    ev = epool.tile([128, nt], BF16, tag="ev")
                    nc.scalar.activation(ev[:], ps[:], AF.Identity, bias=biz_sb[:, m:m + 1])
                    nc.sync.dma_start(xi[:, m, :], ev[:])

            # ---------------- phase B: recurrence ----------------
            nc.vector.memset(h32[:], 0.0)
            nc.vector.memset(h16[:], 0.0)
            last = l == NLAYERS - 1
            if not last:
                hseq = hqpool.tile([128, NK, nt], BF16, tag="hseq")

            with tc.For_i(0, nblk) as blk:
                coff = blk * (BLK * BC)
                xiba = xipool.tile([128, NM, BLKA * BC], BF16, tag="xiba")
                nc.sync.dma_start(xiba[:], xi[:, :, bass.ds(coff, BLKA * BC)])
                xibb = xipool.tile([128, NM, (BLK - BLKA) * BC], BF16, tag="xibb")
                nc.sync.dma_start(xibb[:], xi[:, :, bass.ds(coff + BLKA * BC,
                                                            (BLK - BLKA) * BC)])
                if not last:
                    winb = wnpool.tile([128, NK, BLK * BC], BF16, tag="win")
                for dt in range(BLK):
                    pg = pgpool.tile([128, NM, BC], F32, tag="pg")
                    for m in range(NM):
                        for k in range(NK):
                            nc.tensor.matmul(pg[:, m, :],
                                             whh_sb[:, k, m * 128:(m + 1) * 128],
                                             h16[:, k, :],
                                             start=(k == 0), stop=(k == NK - 1))
                    if dt < BLKA:
                        xs = xiba[:, :, dt * BC:(dt + 1) * BC]
                    else:
                        xs = xibb[:, :, (dt - BLKA) * BC:(dt - BLKA + 1) * BC]
                    rzp = tpool.tile([128, 256], F32, tag="rzp")
                    nc.vector.tensor_add(rzp[:], pg[:, 0:16, :], xs[:, 0:16, :])
                    rzs = tpool.tile([128, 256], F32, tag="rzs")
                    nc.scalar.activation(rzs[:], rzp[:], AF.Tanh, scale=0.5)
                    rz = tpool.tile([128, 256], F32, tag="rz")
                    nc.vector.tensor_scalar(rz[:], rzs[:], 0.5, 0.5,
                                            mybir.AluOpType.mult,
                                            mybir.AluOpType.add)
                    hnb = tpool.tile([128, 128], F32, tag="hnb")
                    nc.vector.tensor_add(hnb[:], pg[:, 16:24, :], bhn_sb[:])
                    t1 = tpool.tile([128, 128], F32, tag="t1")
                    nc.vector.tensor_mul(t1[:], rz[:, 0:128], hnb[:])
                    t2 = tpool.tile([128, 128], F32, tag="t2")
                    nc.vector.tensor_add(t2[:], t1[:], xs[:, 16:24, :])
                    nt_ = tpool.tile([128, 128], F32, tag="nt")
                    nc.scalar.activation(nt_[:], t2[:], AF.Tanh)
                    d = tpool.tile([128, 128], F32, tag="d")
                    nc.vector.tensor_sub(d[:], h32[:], nt_[:])
                    t4 = tpool.tile([128, 128], F32, tag="t4")
                    nc.vector.tensor_mul(t4[:], d[:], rz[:, 128:256])
                    nc.vector.tensor_add(h16[:], t4[:], nt_[:])
                    nc.vector.tensor_add(h32[:], t4[:], nt_[:])
                    if not last:
                        nc.vector.tensor_copy(winb[:, :, dt * BC:(dt + 1) * BC], h16[:])
                if not last:
                    nc.sync.dma_start(hseq[:, :, bass.ds(coff, BLK * BC)], winb[:])
            if not last:
                hseq_prev = hseq

        # ---------------- final FC ----------------
        psfc = fcpool.tile([PRED, BC], F32, tag="psfc")
        for k in range(NK):
            nc.tensor.matmul(psfc[:], fcw_sb[:, k, :], h16[:, k, :],
                             start=(k == 0), stop=(k == NK - 1))
        ofc = epool.tile([PRED, BC], F32, tag="ofc")
        nc.scalar.activation(ofc[:], psfc[:], AF.Identity, bias=fcb_sb[:])
        nc.sync.dma_start(out[:], ofc[:])

    nc.compile()
    return nc


def prep_inputs(inputs, T_=T):
    """Host-side layout preprocessing. Returns (in_maps, shared)."""
    x = np.asarray(inputs["x"], np.float32)

    def chunkT(w):  # [G_out, K*128] -> [128, K, G_out] (lhsT layout)
        w = np.asarray(w, np.float32)
        gout, kin = w.shape
        return np.ascontiguousarray(
            w.T.reshape(kin // 128, 128, gout).transpose(1, 0, 2)
        ).astype(np_bf16)

    shared = {}
    for l in range(NLAYERS):
        wih = np.asarray(inputs[f"w_ih_{l}"], np.float32)
        shared[f"wih{l}"] = chunkT(wih)
        shared[f"whh{l}"] = chunkT(inputs[f"w_hh_{l}"])
        b_ih = np.asarray(inputs[f"b_ih_{l}"], np.float32)
        b_hh = np.asarray(inputs[f"b_hh_{l}"], np.float32)
        comb = b_ih.copy()
        comb[:2 * H] += b_hh[:2 * H]
        shared[f"biz{l}"] = np.ascontiguousarray(comb.reshape(NM, 128).T)
        bhn = b_hh[2 * H:].reshape(NK, 128).T  # [128, NK]
        shared[f"bhn{l}"] = np.ascontiguousarray(np.repeat(bhn, BC, axis=1))
    shared["fcw"] = chunkT(inputs["fc_w"])
    shared["fcb"] = np.asarray(inputs["fc_b"], np.float32).reshape(PRED, 1)

    in_maps = []
    for c in range(NCORES):
        xc = x[c * BC:(c + 1) * BC, :T_, :]  # [BC, T, IN]
        xTc = np.ascontiguousarray(
            xc.transpose(2, 1, 0).reshape(IN, T_ * BC)
        ).astype(np_bf16)
        m = dict(shared)
        m["xT"] = xTc
        in_maps.append(m)
    return in_maps


_NC_CACHE = {}


def kernel(**inputs):
    if "nc" not in _NC_CACHE:
        _NC_CACHE["nc"] = build()
    nc = _NC_CACHE["nc"]
    in_maps = prep_inputs(inputs)
    res = run_bass_kernel_spmd(nc, in_maps, list(range(NCORES)))
    outs = []
    for c in range(NCORES):
        o = np.asarray(res.results[c]["out"], np.float32)  # [PRED, BC]
        outs.append(o.T)  # [BC, PRED]
    return np.concatenate(outs, axis=0)  # [B, PRED]


if __name__ == "__main__":
    rng = np.random.default_rng(0)
    k = 1.0 / np.sqrt(H)
    ins = {"x": rng.standard_normal((B, T, IN), dtype=np.float32)}
    for l in range(NLAYERS):
        ind = IN if l == 0 else H
        ins[f"w_ih_{l}"] = rng.uniform(-k, k, (G, ind)).astype(np.float32)
        ins[f"w_hh_{l}"] = rng.uniform(-k, k, (G, H)).astype(np.float32)
        ins[f"b_ih_{l}"] = rng.uniform(-k, k, (G,)).astype(np.float32)
        ins[f"b_hh_{l}"] = rng.uniform(-k, k, (G,)).astype(np.float32)
    ins["fc_w"] = rng.uniform(-k, k, (PRED, H)).astype(np.float32)
    ins["fc_b"] = rng.uniform(-k, k, (PRED,)).astype(np.float32)
    print(kernel(**ins).shape)



# revision 12
# speedup vs baseline: 1.1074x; 1.1074x over previous
"""Trainium2 Bass kernel for a 3-layer GRU (B=128, T=512, IN=128, H=1024, PRED=96).

Strategy: data-parallel over batch across 8 NeuronCores (B_core=16).
Per core, per layer:
  phase A (inproj): xi = W_ih @ h_prev_seq as batched N=512 GEMMs (bf16),
      biases (b_ih, + b_hh for r/z gates) folded into the PSUM->SBUF
      evacuation on the scalar engine; xi round-trips through DRAM.
  phase B (recurrence): 512 sequential steps; per step 192 self-loading
      bf16 matmuls (K=128, M=128, N=16) accumulate W_hh @ h_t into one
      PSUM bank laid out gate-major [128, 24 chunks x 16 batch]; gate
      math on DVE/ACT in the same layout; hidden state kept fp32 with a
      bf16 shadow for the tensor engine.
Final FC done on-chip; host only transposes [96,16] -> [16,96] per core.
"""

import numpy as np
from contextlib import ExitStack

import concourse.bass as bass
import concourse.bacc as bacc
import concourse.mybir as mybir
import concourse.tile as tile
from concourse.bass_utils import run_bass_kernel_spmd

try:
    from ml_dtypes import bfloat16 as np_bf16
    from ml_dtypes import float8_e3m4 as np_fp8
except ImportError:  # pragma: no cover
    import jax.numpy as jnp

    np_bf16 = jnp.bfloat16
    np_fp8 = jnp.float8_e3m4

B, T, IN, H, NLAYERS, PRED = 128, 512, 128, 1024, 3, 96
NCORES = 8
BC = B // NCORES  # 16
G = 3 * H  # 3072
NK = H // 128  # 8
NM = G // 128  # 24
BLK = 16  # recurrence steps per For_i iteration
BLKA = 4  # early sub-block of xi steps
F32, BF16 = mybir.dt.float32, mybir.dt.bfloat16
FP8 = mybir.dt.float8e3
SCL = 256.0  # W_hh fp8 scale; gates are computed at SCL x and descaled in tanh
AF = mybir.ActivationFunctionType


def build(T_=T):
    nt = T_ * BC
    nblk = T_ // BLK
    ntile = nt // 512 if nt >= 512 else 0
    nc = bacc.Bacc("TRN2", target_bir_lowering=False, debug=False,
                   num_devices=NCORES)

    xT = nc.dram_tensor("xT", [128, nt], BF16, kind="ExternalInput")
    wihs = [
        nc.dram_tensor("wih0", [128, 1, G], BF16, kind="ExternalInput"),
        nc.dram_tensor("wih1", [128, NK, G], BF16, kind="ExternalInput"),
        nc.dram_tensor("wih2", [128, NK, G], BF16, kind="ExternalInput"),
    ]
    whhs = [nc.dram_tensor(f"whh{l}", [128, NK, G], FP8, kind="ExternalInput")
            for l in range(NLAYERS)]
    bizs = [nc.dram_tensor(f"biz{l}", [128, NM], F32, kind="ExternalInput")
            for l in range(NLAYERS)]
    bhns = [nc.dram_tensor(f"bhn{l}", [128, NK * BC], F32, kind="ExternalInput")
            for l in range(NLAYERS)]
    fcw = nc.dram_tensor("fcw", [128, NK, PRED], BF16, kind="ExternalInput")
    fcb = nc.dram_tensor("fcb", [PRED, 1], F32, kind="ExternalInput")
    out = nc.dram_tensor("out", [PRED, BC], F32, kind="ExternalOutput")

    with tile.TileContext(nc) as tc, ExitStack() as ctx:
        wpool = ctx.enter_context(tc.tile_pool(name="w", bufs=1))
        cpool = ctx.enter_context(tc.tile_pool(name="const", bufs=1))
        xpool = ctx.enter_context(tc.tile_pool(name="xT", bufs=1))
        rpool = ctx.enter_context(tc.tile_pool(name="rhs", bufs=2))
        epool = ctx.enter_context(tc.tile_pool(name="ev", bufs=3))
        xipool = ctx.enter_context(tc.tile_pool(name="xib", bufs=2))
        wnpool = ctx.enter_context(tc.tile_pool(name="win", bufs=2))
        tpool = ctx.enter_context(tc.tile_pool(name="tmp", bufs=2))
        spool = ctx.enter_context(tc.tile_pool(name="state", bufs=1))
        pspool = ctx.enter_context(tc.tile_pool(name="ps", bufs=3, space="PSUM"))
        fcpool = ctx.enter_context(tc.tile_pool(name="psfc", bufs=1, space="PSUM"))
        pgpool = ctx.enter_context(tc.tile_pool(name="pg", bufs=1, space="PSUM"))
        dpool = ctx.enter_context(tc.tile_pool(name="dram", bufs=1, space="DRAM"))
        hqpool = ctx.enter_context(tc.tile_pool(name="hseq", bufs=2, space="DRAM"))

        # persistent state
        h32 = spool.tile([128, NK, BC], F32, tag="h32")
        h16 = spool.tile([128, NK, BC], BF16, tag="h16")

        xT_sb = xpool.tile([128, nt], BF16, tag="xT")
        nc.sync.dma_start(xT_sb[:], xT[:])
        fcw_sb = cpool.tile([128, NK, PRED], BF16, tag="fcw")
        nc.sync.dma_start(fcw_sb[:], fcw[:])
        fcb_sb = cpool.tile([PRED, 1], F32, tag="fcb")
        nc.sync.dma_start(fcb_sb[:], fcb[:])

        hseq_prev = None
        for l in range(NLAYERS):
            nkl = 1 if l == 0 else NK
            wih_sb = wpool.tile([128, NK, G], BF16, tag="wih")
            nc.sync.dma_start(wih_sb[:, 0:nkl, :], wihs[l][:])
            whh_sb = wpool.tile([128, NK, G], FP8, tag="whh")
            nc.sync.dma_start(whh_sb[:], whhs[l][:])
            biz_sb = cpool.tile([128, NM], F32, tag="biz")
            nc.sync.dma_start(biz_sb[:], bizs[l][:])
            bhn_sb = cpool.tile([128, NK * BC], F32, tag="bhn")
            nc.sync.dma_start(bhn_sb[:], bhns[l][:])

            xi = dpool.tile([128, NM, nt], BF16, tag="xi")

            # ---------------- phase A: input projection ----------------
            def inproj_body(n):
                noff = n * 512
                if l == 0:
                    rhs_of = [xT_sb[:, bass.ds(noff, 512)]]
                else:
                    rhsb = rpool.tile([128, NK, 512], BF16, tag="rhs")
                    nc.sync.dma_start(rhsb[:], hseq_prev[:, :, bass.ds(noff, 512)])
                    rhs_of = [rhsb[:, k, :] for k in range(NK)]
                for m in range(NM):
                    ps = pspool.tile([128, 512], F32, tag="ps")
                    for k in range(nkl):
                        nc.tensor.matmul(ps[:], wih_sb[:, k, m * 128:(m + 1) * 128],
                                         rhs_of[k], start=(k == 0), stop=(k == nkl - 1))
                    ev = epool.tile([128, 512], BF16, tag="ev")
                    nc.scalar.activation(ev[:], ps[:], AF.Identity,
                                         bias=biz_sb[:, m:m + 1], scale=SCL)
                    nc.sync.dma_start(xi[:, m, bass.ds(noff, 512)], ev[:])

            if ntile:
                with tc.For_i(0, ntile) as n:
                    inproj_body(n)
            else:  # tiny-T debug path
                for m in range(NM):
                    ps = pspool.tile([128, nt], F32, tag="ps")
                    if l == 0:
                        for k in range(1):
                            nc.tensor.matmul(ps[:], wih_sb[:, k, m * 128:(m + 1) * 128],
                                             xT_sb[:], start=True, stop=True)
                    else:
                        rhsb = rpool.tile([128, NK, nt], BF16, tag="rhs")
                        nc.sync.dma_start(rhsb[:], hseq_prev[:])
                        for k in range(NK):
                            nc.tensor.matmul(ps[:], wih_sb[:, k, m * 128:(m + 1) * 128],
                                             rhsb[:, k, :], start=(k == 0), stop=(k == NK - 1))
                    ev = epool.tile([128, nt], BF16, tag="ev")
                    nc.scalar.activation(ev[:], ps[:], AF.Identity,
                                         bias=biz_sb[:, m:m + 1], scale=SCL)
                    nc.sync.dma_start(xi[:, m, :], ev[:])

            # ---------------- phase B: recurrence ----------------
            nc.vector.memset(h32[:], 0.0)
            nc.vector.memset(h16[:], 0.0)
            last = l == NLAYERS - 1
            if not last:
                hseq = hqpool.tile([128, NK, nt], BF16, tag="hseq")

            with tc.For_i(0, nblk) as blk:
                coff = blk * (BLK * BC)
                xiba = xipool.tile([128, NM, BLKA * BC], BF16, tag="xiba")
                nc.sync.dma_start(xiba[:], xi[:, :, bass.ds(coff, BLKA * BC)])
                xibb = xipool.tile([128, NM, (BLK - BLKA) * BC], BF16, tag="xibb")
                nc.sync.dma_start(xibb[:], xi[:, :, bass.ds(coff + BLKA * BC,
                                                            (BLK - BLKA) * BC)])
                if not last:
                    winb = wnpool.tile([128, NK, BLK * BC], BF16, tag="win")
                for dt in range(BLK):
                    # gate order: z (m 8..15), r (m 0..7), n (m 16..23) so the
                    # z/r sigmoid chains overlap the remaining matmul stream
                    pgz = pgpool.tile([128, NK, BC], F32, tag="pgz")
                    pgr = pgpool.tile([128, NK, BC], F32, tag="pgr")
                    pgn = pgpool.tile([128, NK, BC], F32, tag="pgn")
                    for m in range(8):
                        for k in range(NK):
                            nc.tensor.matmul(pgz[:, m, :],
                                             whh_sb[:, k, (m + 8) * 128:(m + 9) * 128],
                                             h16[:, k, :],
                                             start=(k == 0), stop=(k == NK - 1))
                    for m in range(8):
                        for k in range(NK):
                            nc.tensor.matmul(pgr[:, m, :],
                                             whh_sb[:, k, m * 128:(m + 1) * 128],
                                             h16[:, k, :],
                                             start=(k == 0), stop=(k == NK - 1))
                    for m in range(8):
                        for k in range(NK):
                            nc.tensor.matmul(pgn[:, m, :],
                                             whh_sb[:, k, (m + 16) * 128:(m + 17) * 128],
                                             h16[:, k, :],
                                             start=(k == 0), stop=(k == NK - 1))
                    if dt < BLKA:
                        xs = xiba[:, :, dt * BC:(dt + 1) * BC]
                    else:
                        xs = xibb[:, :, (dt - BLKA) * BC:(dt - BLKA + 1) * BC]
                    # ---- z chain (overlaps r+n matmuls) ----
                    zp = tpool.tile([128, 128], F32, tag="zp")
                    nc.vector.tensor_add(zp[:], pgz[:], xs[:, 8:16, :])
                    zt = tpool.tile([128, 128], F32, tag="zt")
                    nc.scalar.activation(zt[:], zp[:], AF.Tanh, scale=0.5 / SCL)
                    z = tpool.tile([128, 128], F32, tag="z")
                    nc.vector.tensor_scalar(z[:], zt[:], 0.5, 0.5,
                                            mybir.AluOpType.mult,
                                            mybir.AluOpType.add)
                    zh = tpool.tile([128, 128], F32, tag="zh")
                    nc.vector.tensor_mul(zh[:], z[:], h32[:])
                    zc = tpool.tile([128, 128], F32, tag="zc")
                    nc.vector.tensor_scalar(zc[:], zt[:], -0.5, 0.5,
                                            mybir.AluOpType.mult,
                                            mybir.AluOpType.add)
                    # ---- r chain (overlaps n matmuls) ----
                    rp = tpool.tile([128, 128], F32, tag="rp")
                    nc.vector.tensor_add(rp[:], pgr[:], xs[:, 0:8, :])
                    rt = tpool.tile([128, 128], F32, tag="rt")
                    nc.scalar.activation(rt[:], rp[:], AF.Tanh, scale=0.5 / SCL)
                    r = tpool.tile([128, 128], F32, tag="r")
                    nc.vector.tensor_scalar(r[:], rt[:], 0.5, 0.5,
                                            mybir.AluOpType.mult,
                                            mybir.AluOpType.add)
                    # ---- n chain (the serial tail) ----
                    hnb = tpool.tile([128, 128], F32, tag="hnb")
                    nc.vector.tensor_add(hnb[:], pgn[:], bhn_sb[:])
                    t1 = tpool.tile([128, 128], F32, tag="t1")
                    nc.vector.tensor_mul(t1[:], r[:], hnb[:])
                    t2 = tpool.tile([128, 128], F32, tag="t2")
                    nc.vector.tensor_add(t2[:], t1[:], xs[:, 16:24, :])
                    nt_ = tpool.tile([128, 128], F32, tag="nt")
                    nc.scalar.activation(nt_[:], t2[:], AF.Tanh, scale=1.0 / SCL)
                    p1 = tpool.tile([128, 128], F32, tag="p1")
                    nc.vector.tensor_mul(p1[:], zc[:], nt_[:])
                    # h16 first: the next step's matmuls wait only on it
                    nc.vector.tensor_add(h16[:], p1[:], zh[:])
                    if not last:
                        nc.vector.tensor_add(
                            winb[:, :, dt * BC:(dt + 1) * BC], p1[:], zh[:])
                    nc.vector.tensor_add(h32[:], p1[:], zh[:])
                if not last:
                    nc.sync.dma_start(hseq[:, :, bass.ds(coff, BLK * BC)], winb[:])
            if not last:
                hseq_prev = hseq

        # ---------------- final FC ----------------
        psfc = fcpool.tile([PRED, BC], F32, tag="psfc")
        for k in range(NK):
            nc.tensor.matmul(psfc[:], fcw_sb[:, k, :], h16[:, k, :],
                             start=(k == 0), stop=(k == NK - 1))
        ofc = epool.tile([PRED, BC], F32, tag="ofc")
        nc.scalar.activation(ofc[:], psfc[:], AF.Identity, bias=fcb_sb[:])
        nc.sync.dma_start(out[:], ofc[:])

    nc.compile()
    return nc


def prep_inputs(inputs, T_=T):
    """Host-side layout preprocessing. Returns (in_maps, shared)."""
    x = np.asarray(inputs["x"], np.float32)

    def chunkT(w):  # [G_out, K*128] -> [128, K, G_out] (lhsT layout)
        w = np.asarray(w, np.float32)
        gout, kin = w.shape
        return np.ascontiguousarray(
            w.T.reshape(kin // 128, 128, gout).transpose(1, 0, 2)
        ).astype(np_bf16)

    def chunkT_fp8(w):  # [G_out, K*128] -> [128, K, G_out] fp8, pre-scaled
        w = np.asarray(w, np.float32) * SCL
        gout, kin = w.shape
        return np.ascontiguousarray(
            w.T.reshape(kin // 128, 128, gout).transpose(1, 0, 2)
        ).astype(np_fp8)

    shared = {}
    for l in range(NLAYERS):
        wih = np.asarray(inputs[f"w_ih_{l}"], np.float32)
        shared[f"wih{l}"] = chunkT(wih)
        shared[f"whh{l}"] = chunkT_fp8(inputs[f"w_hh_{l}"])
        b_ih = np.asarray(inputs[f"b_ih_{l}"], np.float32)
        b_hh = np.asarray(inputs[f"b_hh_{l}"], np.float32)
        comb = b_ih.copy()
        comb[:2 * H] += b_hh[:2 * H]
        shared[f"biz{l}"] = np.ascontiguousarray(comb.reshape(NM, 128).T) * SCL
        bhn = b_hh[2 * H:].reshape(NK, 128).T  # [128, NK]
        shared[f"bhn{l}"] = np.ascontiguousarray(np.repeat(bhn, BC, axis=1)) * SCL
    shared["fcw"] = chunkT(inputs["fc_w"])
    shared["fcb"] = np.asarray(inputs["fc_b"], np.float32).reshape(PRED, 1)

    in_maps = []
    for c in range(NCORES):
        xc = x[c * BC:(c + 1) * BC, :T_, :]  # [BC, T, IN]
        xTc = np.ascontiguousarray(
            xc.transpose(2, 1, 0).reshape(IN, T_ * BC)
        ).astype(np_bf16)
        m = dict(shared)
        m["xT"] = xTc
        in_maps.append(m)
    return in_maps


_NC_CACHE = {}


def kernel(**inputs):
    if "nc" not in _NC_CACHE:
        _NC_CACHE["nc"] = build()
    nc = _NC_CACHE["nc"]
    in_maps = prep_inputs(inputs)
    res = run_bass_kernel_spmd(nc, in_maps, list(range(NCORES)))
    outs = []
    for c in range(NCORES):
        o = np.asarray(res.results[c]["out"], np.float32)  # [PRED, BC]
        outs.append(o.T)  # [BC, PRED]
    return np.concatenate(outs, axis=0)  # [B, PRED]


if __name__ == "__main__":
    rng = np.random.default_rng(0)
    k = 1.0 / np.sqrt(H)
    ins = {"x": rng.standard_normal((B, T, IN), dtype=np.float32)}
    for l in range(NLAYERS):
        ind = IN if l == 0 else H
        ins[f"w_ih_{l}"] = rng.uniform(-k, k, (G, ind)).astype(np.float32)
        ins[f"w_hh_{l}"] = rng.uniform(-k, k, (G, H)).astype(np.float32)
        ins[f"b_ih_{l}"] = rng.uniform(-k, k, (G,)).astype(np.float32)
        ins[f"b_hh_{l}"] = rng.uniform(-k, k, (G,)).astype(np.float32)
    ins["fc_w"] = rng.uniform(-k, k, (PRED, H)).astype(np.float32)
    ins["fc_b"] = rng.uniform(-k, k, (PRED,)).astype(np.float32)
    print(kernel(**ins).shape)



# revision 20
# speedup vs baseline: 1.2104x; 1.0930x over previous
"""Trainium2 Bass kernel for a 3-layer GRU (B=128, T=512, IN=128, H=1024, PRED=96).

Strategy: data-parallel over batch across 8 NeuronCores (B_core=16).

Layer l's recurrence (phase B) runs 512 sequential steps; per step 192
bf16 matmuls (K=128, M=128, N=16) accumulate W_hh @ h_t into three PSUM
tiles (z, r, n gate groups, issued in that order) so the z/r sigmoid
chains on DVE/ACT overlap the remaining matmul stream; only the n-gate
tail (r*pg_n -> +rb -> tanh -> combine) is serial per step.

Layer l+1's input projection is absorbed into layer l's recurrence
loop: after each step's matmuls, 1-2 m-chunks of W_ih_{l+1} @ hseq_l
(N=256 GEMMs over the previous block) are issued on the tensor queue.
They execute inside the per-step gate-math tail (otherwise PE-idle) and
keep the PE array busy enough that the HAM clock gate stays at 2.4 GHz.
Layer 0's input projection runs standalone up front (reads xT).

Weights are pre-scaled by SCL host-side; gates are computed at SCL*x
and descaled inside the tanh activations (free scale parameter).
Final FC done on-chip; host only transposes [96,16] -> [16,96] per core.
"""

import numpy as np
from contextlib import ExitStack

import concourse.bass as bass
import concourse.bacc as bacc
import concourse.mybir as mybir
import concourse.tile as tile
from concourse.bass_utils import run_bass_kernel_spmd

try:
    from ml_dtypes import bfloat16 as np_bf16
except ImportError:  # pragma: no cover
    import jax.numpy as jnp

    np_bf16 = jnp.bfloat16

B, T, IN, H, NLAYERS, PRED = 128, 512, 128, 1024, 3, 96
NCORES = 8
BC = B // NCORES  # 16
G = 3 * H  # 3072
NK = H // 128  # 8
NM = G // 128  # 24
BLK = 16  # recurrence steps per For_i iteration
BLKA = 4  # early sub-block of xi steps
CB = BLK * BC  # columns per block (256)
F32, BF16 = mybir.dt.float32, mybir.dt.bfloat16
SCL = 256.0  # weight pre-scale; descaled in the tanh activations
AF = mybir.ActivationFunctionType


def build(T_=T):
    nt = T_ * BC
    nblk = T_ // BLK
    ntile = nt // 512 if nt >= 512 else 1
    nc = bacc.Bacc("TRN2", target_bir_lowering=False, debug=False,
                   num_devices=NCORES)

    xT = nc.dram_tensor("xT", [128, nt], BF16, kind="ExternalInput")
    wihs = [
        nc.dram_tensor("wih0", [128, 1, G], BF16, kind="ExternalInput"),
        nc.dram_tensor("wih1", [128, NK, G], BF16, kind="ExternalInput"),
        nc.dram_tensor("wih2", [128, NK, G], BF16, kind="ExternalInput"),
    ]
    whhs = [nc.dram_tensor(f"whh{l}", [128, NK, G], BF16, kind="ExternalInput")
            for l in range(NLAYERS)]
    bizs = [nc.dram_tensor(f"biz{l}", [128, NM], F32, kind="ExternalInput")
            for l in range(NLAYERS)]
    bhns = [nc.dram_tensor(f"bhn{l}", [128, NK * BC], F32, kind="ExternalInput")
            for l in range(NLAYERS)]
    fcw = nc.dram_tensor("fcw", [128, NK, PRED], BF16, kind="ExternalInput")
    fcb = nc.dram_tensor("fcb", [PRED, 1], F32, kind="ExternalInput")
    out = nc.dram_tensor("out", [PRED, BC], F32, kind="ExternalOutput")

    with tile.TileContext(nc) as tc, ExitStack() as ctx:
        wpool = ctx.enter_context(tc.tile_pool(name="w", bufs=1))
        cpool = ctx.enter_context(tc.tile_pool(name="const", bufs=1))
        xpool = ctx.enter_context(tc.tile_pool(name="xT", bufs=1))
        rpool = ctx.enter_context(tc.tile_pool(name="rhs", bufs=2))
        epool = ctx.enter_context(tc.tile_pool(name="ev", bufs=3))
        xipool = ctx.enter_context(tc.tile_pool(name="xib", bufs=2))
        wnpool = ctx.enter_context(tc.tile_pool(name="win", bufs=2))
        tpool = ctx.enter_context(tc.tile_pool(name="tmp", bufs=2))
        spool = ctx.enter_context(tc.tile_pool(name="state", bufs=1))
        pspool = ctx.enter_context(tc.tile_pool(name="ps", bufs=2, space="PSUM"))
        fcpool = ctx.enter_context(tc.tile_pool(name="psfc", bufs=1, space="PSUM"))
        pgpool = ctx.enter_context(tc.tile_pool(name="pg", bufs=1, space="PSUM"))
        dpool = ctx.enter_context(tc.tile_pool(name="dram", bufs=1, space="DRAM"))
        hqpool = ctx.enter_context(tc.tile_pool(name="hseq", bufs=2, space="DRAM"))

        # persistent state
        h32 = spool.tile([128, NK, BC], F32, tag="h32")
        h16 = spool.tile([128, NK, BC], BF16, tag="h16")

        xT_sb = xpool.tile([128, nt], BF16, tag="xT")
        nc.sync.dma_start(xT_sb[:], xT[:])
        fcw_sb = cpool.tile([128, NK, PRED], BF16, tag="fcw")
        nc.sync.dma_start(fcw_sb[:], fcw[:])
        fcb_sb = cpool.tile([PRED, 1], F32, tag="fcb")
        nc.sync.dma_start(fcb_sb[:], fcb[:])
        biz_sb = []
        bhn_sb = []
        for l in range(NLAYERS):
            t = cpool.tile([128, NM], F32, tag=f"biz{l}")
            nc.sync.dma_start(t[:], bizs[l][:])
            biz_sb.append(t)
            t = cpool.tile([128, NK * BC], F32, tag=f"bhn{l}")
            nc.sync.dma_start(t[:], bhns[l][:])
            bhn_sb.append(t)

        # ---------------- layer 0 input projection (standalone) ----------------
        wih_sb = wpool.tile([128, NK, G], BF16, tag="wih")
        nc.sync.dma_start(wih_sb[:, 0:1, :], wihs[0][:])
        xi = dpool.tile([128, NM, nt], BF16, tag="xi0")

        def l0_inproj(noff, ncols):
            for m in range(NM):
                ps = pspool.tile([128, 512], F32, tag="ps")
                nc.tensor.matmul(ps[:, 0:ncols], wih_sb[:, 0, m * 128:(m + 1) * 128],
                                 xT_sb[:, bass.ds(noff, ncols)], start=True, stop=True)
                ev = epool.tile([128, 512], BF16, tag="ev")
                nc.scalar.activation(ev[:, 0:ncols], ps[:, 0:ncols], AF.Identity,
                                     bias=biz_sb[0][:, m:m + 1], scale=SCL)
                nc.sync.dma_start(xi[:, m, bass.ds(noff, ncols)], ev[:, 0:ncols])

        if nt >= 512:
            with tc.For_i(0, ntile) as n:
                l0_inproj(n * 512, 512)
        else:
            l0_inproj(0, nt)

        # ---------------- layers ----------------
        for l in range(NLAYERS):
            last = l == NLAYERS - 1
            whh_sb = wpool.tile([128, NK, G], BF16, tag="whh")
            nc.sync.dma_start(whh_sb[:], whhs[l][:])
            if not last:
                wih_sb = wpool.tile([128, NK, G], BF16, tag="wih")
                nc.sync.dma_start(wih_sb[:], wihs[l + 1][:])
                hseq = hqpool.tile([128, NK, nt], BF16, tag="hseq")
                xi_next = dpool.tile([128, NM, nt], BF16, tag=f"xi{(l + 1) % 2}")

            nc.vector.memset(h32[:], 0.0)
            nc.vector.memset(h16[:], 0.0)

            def inproj_unit(m, src_off, dst_off):
                """One m-chunk of layer l+1's inproj over one CB-col block."""
                psI = pspool.tile([128, CB], F32, tag="psI")
                for k in range(NK):
                    nc.tensor.matmul(psI[:], wih_sb[:, k, m * 128:(m + 1) * 128],
                                     rhsI[:, k, :], start=(k == 0), stop=(k == NK - 1))
                evI = epool.tile([128, CB], BF16, tag="evI")
                nc.scalar.activation(evI[:], psI[:], AF.Identity,
                                     bias=biz_sb[l + 1][:, m:m + 1], scale=SCL)
                nc.sync.dma_start(xi_next[:, m, bass.ds(dst_off, CB)], evI[:])

            def step(dt, xiba, xibb, winb, inp_off):
                # gate matmuls: z (m 8..15), r (m 0..7), n (m 16..23)
                pgz = pgpool.tile([128, NK, BC], F32, tag="pgz")
                pgr = pgpool.tile([128, NK, BC], F32, tag="pgr")
                pgn = pgpool.tile([128, NK, BC], F32, tag="pgn")
                for m in range(8):
                    for k in range(NK):
                        nc.tensor.matmul(pgz[:, m, :],
                                         whh_sb[:, k, (m + 8) * 128:(m + 9) * 128],
                                         h16[:, k, :],
                                         start=(k == 0), stop=(k == NK - 1))
                for m in range(8):
                    for k in range(NK):
                        nc.tensor.matmul(pgr[:, m, :],
                                         whh_sb[:, k, m * 128:(m + 1) * 128],
                                         h16[:, k, :],
                                         start=(k == 0), stop=(k == NK - 1))
                for m in range(8):
                    for k in range(NK):
                        nc.tensor.matmul(pgn[:, m, :],
                                         whh_sb[:, k, (m + 16) * 128:(m + 17) * 128],
                                         h16[:, k, :],
                                         start=(k == 0), stop=(k == NK - 1))
                if dt < BLKA:
                    xs = xiba[:, :, dt * BC:(dt + 1) * BC]
                else:
                    xs = xibb[:, :, (dt - BLKA) * BC:(dt - BLKA + 1) * BC]
                # ---- z chain (overlaps r+n matmuls) ----
                zp = tpool.tile([128, 128], F32, tag="zp")
                nc.vector.tensor_add(zp[:], pgz[:], xs[:, 8:16, :])
                zt = tpool.tile([128, 128], F32, tag="zt")
                nc.scalar.activation(zt[:], zp[:], AF.Tanh, scale=0.5 / SCL)
                z = tpool.tile([128, 128], F32, tag="z")
                nc.vector.tensor_scalar(z[:], zt[:], 0.5, 0.5,
                                        mybir.AluOpType.mult,
                                        mybir.AluOpType.add)
                zh = tpool.tile([128, 128], F32, tag="zh")
                nc.vector.tensor_mul(zh[:], z[:], h32[:])
                zc = tpool.tile([128, 128], F32, tag="zc")
                nc.vector.tensor_scalar(zc[:], zt[:], -0.5, 0.5,
                                        mybir.AluOpType.mult,
                                        mybir.AluOpType.add)
                # ---- r chain (overlaps n matmuls) ----
                rp = tpool.tile([128, 128], F32, tag="rp")
                nc.vector.tensor_add(rp[:], pgr[:], xs[:, 0:8, :])
                rt = tpool.tile([128, 128], F32, tag="rt")
                nc.scalar.activation(rt[:], rp[:], AF.Tanh, scale=0.5 / SCL)
                r = tpool.tile([128, 128], F32, tag="r")
                nc.vector.tensor_scalar(r[:], rt[:], 0.5, 0.5,
                                        mybir.AluOpType.mult,
                                        mybir.AluOpType.add)
                # rb = r*b_hn + xs_n, precomputed off the critical tail
                rb1 = tpool.tile([128, 128], F32, tag="rb1")
                nc.vector.tensor_mul(rb1[:], r[:], bhn_sb[l][:])
                rb2 = tpool.tile([128, 128], F32, tag="rb2")
                nc.vector.tensor_add(rb2[:], rb1[:], xs[:, 16:24, :])
                # ---- n chain (the serial tail) ----
                t1 = tpool.tile([128, 128], F32, tag="t1")
                nc.vector.tensor_mul(t1[:], r[:], pgn[:])
                t2 = tpool.tile([128, 128], F32, tag="t2")
                nc.vector.tensor_add(t2[:], t1[:], rb2[:])
                nt_ = tpool.tile([128, 128], F32, tag="nt")
                nc.scalar.activation(nt_[:], t2[:], AF.Tanh, scale=1.0 / SCL)
                p1 = tpool.tile([128, 128], F32, tag="p1")
                nc.vector.tensor_mul(p1[:], zc[:], nt_[:])
                # h16 first: the next step's matmuls wait only on it
                nc.vector.tensor_add(h16[:], p1[:], zh[:])
                if winb is not None:
                    nc.vector.tensor_add(
                        winb[:, :, dt * BC:(dt + 1) * BC], p1[:], zh[:])
                nc.vector.tensor_add(h32[:], p1[:], zh[:])
                # interleaved next-layer inproj: fills the per-step tail
                # (tensor-queue: after this step's MMs, before the next step's)
                if inp_off is not None:
                    sm = (3 * dt) // 2
                    cnt = (3 * (dt + 1)) // 2 - sm
                    for m in range(sm, sm + cnt):
                        inproj_unit(m, inp_off, inp_off)

            def block(rec_off, inp_off):
                """Recurrence block at column offset rec_off; optionally the
                interleaved next-layer inproj over block at inp_off."""
                xiba = xipool.tile([128, NM, BLKA * BC], BF16, tag="xiba")
                nc.sync.dma_start(xiba[:], xi[:, :, bass.ds(rec_off, BLKA * BC)])
                xibb = xipool.tile([128, NM, (BLK - BLKA) * BC], BF16, tag="xibb")
                nc.sync.dma_start(xibb[:], xi[:, :, bass.ds(rec_off + BLKA * BC,
                                                            (BLK - BLKA) * BC)])
                winb = None
                if not last:
                    winb = wnpool.tile([128, NK, CB], BF16, tag="win")
                for dt in range(BLK):
                    step(dt, xiba, xibb, winb, inp_off)
                if not last:
                    nc.sync.dma_start(hseq[:, :, bass.ds(rec_off, CB)], winb[:])

            if last:
                if nblk > 1:
                    with tc.For_i(0, nblk) as blk:
                        block(blk * CB, None)
                else:
                    block(0, None)
            else:
                rhsI = None
                block(0, None)  # peeled: no previous block to project yet
                if nblk > 1:
                    with tc.For_i(0, nblk - 1) as j:
                        rhsI = rpool.tile([128, NK, CB], BF16, tag="rhsI")
                        nc.sync.dma_start(rhsI[:], hseq[:, :, bass.ds(j * CB, CB)])
                        block((j + 1) * CB, j * CB)
                # trailing inproj for the final block of this layer
                rhsI = rpool.tile([128, NK, CB], BF16, tag="rhsI")
                fin = (nblk - 1) * CB
                nc.sync.dma_start(rhsI[:], hseq[:, :, bass.ds(fin, CB)])
                for m in range(NM):
                    inproj_unit(m, fin, fin)
            if not last:
                xi = xi_next

        # ---------------- final FC ----------------
        psfc = fcpool.tile([PRED, BC], F32, tag="psfc")
        for k in range(NK):
            nc.tensor.matmul(psfc[:], fcw_sb[:, k, :], h16[:, k, :],
                             start=(k == 0), stop=(k == NK - 1))
        ofc = epool.tile([PRED, BC], F32, tag="ofc")
        nc.scalar.activation(ofc[:], psfc[:], AF.Identity, bias=fcb_sb[:])
        nc.sync.dma_start(out[:], ofc[:])

    nc.compile()
    return nc


def prep_inputs(inputs, T_=T):
    """Host-side layout preprocessing. Returns per-core input maps."""
    x = np.asarray(inputs["x"], np.float32)

    def chunkT(w):  # [G_out, K*128] -> [128, K, G_out] (lhsT layout)
        w = np.asarray(w, np.float32)
        gout, kin = w.shape
        return np.ascontiguousarray(
            w.T.reshape(kin // 128, 128, gout).transpose(1, 0, 2)
        ).astype(np_bf16)

    def chunkT_scl(w):  # [G_out, K*128] -> [128, K, G_out], pre-scaled by SCL
        w = np.asarray(w, np.float32) * SCL
        gout, kin = w.shape
        return np.ascontiguousarray(
            w.T.reshape(kin // 128, 128, gout).transpose(1, 0, 2)
        ).astype(np_bf16)

    shared = {}
    for l in range(NLAYERS):
        wih = np.asarray(inputs[f"w_ih_{l}"], np.float32)
        shared[f"wih{l}"] = chunkT(wih)
        shared[f"whh{l}"] = chunkT_scl(inputs[f"w_hh_{l}"])
        b_ih = np.asarray(inputs[f"b_ih_{l}"], np.float32)
        b_hh = np.asarray(inputs[f"b_hh_{l}"], np.float32)
        comb = b_ih.copy()
        comb[:2 * H] += b_hh[:2 * H]
        shared[f"biz{l}"] = np.ascontiguousarray(comb.reshape(NM, 128).T) * SCL
        bhn = b_hh[2 * H:].reshape(NK, 128).T  # [128, NK]
        shared[f"bhn{l}"] = np.ascontiguousarray(np.repeat(bhn, BC, axis=1)) * SCL
    shared["fcw"] = chunkT(inputs["fc_w"])
    shared["fcb"] = np.asarray(inputs["fc_b"], np.float32).reshape(PRED, 1)

    in_maps = []
    for c in range(NCORES):
        xc = x[c * BC:(c + 1) * BC, :T_, :]  # [BC, T, IN]
        xTc = np.ascontiguousarray(
            xc.transpose(2, 1, 0).reshape(IN, T_ * BC)
        ).astype(np_bf16)
        m = dict(shared)
        m["xT"] = xTc
        in_maps.append(m)
    return in_maps


_NC_CACHE = {}


def kernel(**inputs):
    if "nc" not in _NC_CACHE:
        _NC_CACHE["nc"] = build()
    nc = _NC_CACHE["nc"]
    in_maps = prep_inputs(inputs)
    res = run_bass_kernel_spmd(nc, in_maps, list(range(NCORES)))
    outs = []
    for c in range(NCORES):
        o = np.asarray(res.results[c]["out"], np.float32)  # [PRED, BC]
        outs.append(o.T)  # [BC, PRED]
    return np.concatenate(outs, axis=0)  # [B, PRED]


if __name__ == "__main__":
    rng = np.random.default_rng(0)
    k = 1.0 / np.sqrt(H)
    ins = {"x": rng.standard_normal((B, T, IN), dtype=np.float32)}
    for l in range(NLAYERS):
        ind = IN if l == 0 else H
        ins[f"w_ih_{l}"] = rng.uniform(-k, k, (G, ind)).astype(np.float32)
        ins[f"w_hh_{l}"] = rng.uniform(-k, k, (G, H)).astype(np.float32)
        ins[f"b_ih_{l}"] = rng.uniform(-k, k, (G,)).astype(np.float32)
        ins[f"b_hh_{l}"] = rng.uniform(-k, k, (G,)).astype(np.float32)
    ins["fc_w"] = rng.uniform(-k, k, (PRED, H)).astype(np.float32)
    ins["fc_b"] = rng.uniform(-k, k, (PRED,)).astype(np.float32)
    print(kernel(**ins).shape)


# revision 24
# speedup vs baseline: 1.2135x; 1.0025x over previous
"""Trainium2 Bass kernel for a 3-layer GRU (B=128, T=512, IN=128, H=1024, PRED=96).

Strategy: data-parallel over batch across 8 NeuronCores (B_core=16).

Layer l's recurrence (phase B) runs 512 sequential steps; per step 192
bf16 matmuls (K=128, M=128, N=16) accumulate W_hh @ h_t into three PSUM
tiles (z, r, n gate groups, issued in that order) so the z/r sigmoid
chains on DVE/ACT overlap the remaining matmul stream; only the n-gate
tail (r*pg_n -> +rb -> tanh -> combine) is serial per step.

Layer l+1's input projection is absorbed into layer l's recurrence
loop: after each step's matmuls, 1-2 m-chunks of W_ih_{l+1} @ hseq_l
(N=256 GEMMs over the previous block) are issued on the tensor queue.
They execute inside the per-step gate-math tail (otherwise PE-idle) and
keep the PE array busy enough that the HAM clock gate stays at 2.4 GHz.
Layer 0's input projection runs standalone up front (reads xT).

Weights are pre-scaled by SCL host-side; gates are computed at SCL*x
and descaled inside the tanh activations (free scale parameter).
Final FC done on-chip; host only transposes [96,16] -> [16,96] per core.
"""

import numpy as np
from contextlib import ExitStack

import concourse.bass as bass
import concourse.bacc as bacc
import concourse.mybir as mybir
import concourse.tile as tile
from concourse.bass_utils import run_bass_kernel_spmd

try:
    from ml_dtypes import bfloat16 as np_bf16
except ImportError:  # pragma: no cover
    import jax.numpy as jnp

    np_bf16 = jnp.bfloat16

B, T, IN, H, NLAYERS, PRED = 128, 512, 128, 1024, 3, 96
NCORES = 8
BC = B // NCORES  # 16
G = 3 * H  # 3072
NK = H // 128  # 8
NM = G // 128  # 24
BLK = 16  # recurrence steps per For_i iteration
BLKA = 4  # early sub-block of xi steps
CB = BLK * BC  # columns per block (256)
F32, BF16 = mybir.dt.float32, mybir.dt.bfloat16
SCL = 256.0  # weight pre-scale; descaled in the tanh activations
AF = mybir.ActivationFunctionType


def build(T_=T):
    nt = T_ * BC
    nblk = T_ // BLK
    ntile = nt // 512 if nt >= 512 else 1
    nc = bacc.Bacc("TRN2", target_bir_lowering=False, debug=False,
                   num_devices=NCORES)

    xT = nc.dram_tensor("xT", [128, nt], BF16, kind="ExternalInput")
    wihs = [
        nc.dram_tensor("wih0", [128, 1, G], BF16, kind="ExternalInput"),
        nc.dram_tensor("wih1", [128, NK, G], BF16, kind="ExternalInput"),
        nc.dram_tensor("wih2", [128, NK, G], BF16, kind="ExternalInput"),
    ]
    whhs = [nc.dram_tensor(f"whh{l}", [128, NK, G], BF16, kind="ExternalInput")
            for l in range(NLAYERS)]
    bizs = [nc.dram_tensor(f"biz{l}", [128, NM], F32, kind="ExternalInput")
            for l in range(NLAYERS)]
    bhns = [nc.dram_tensor(f"bhn{l}", [128, NK * BC], F32, kind="ExternalInput")
            for l in range(NLAYERS)]
    fcw = nc.dram_tensor("fcw", [128, NK, PRED], BF16, kind="ExternalInput")
    fcb = nc.dram_tensor("fcb", [PRED, 1], F32, kind="ExternalInput")
    out = nc.dram_tensor("out", [PRED, BC], F32, kind="ExternalOutput")

    with tile.TileContext(nc) as tc, ExitStack() as ctx:
        wpool = ctx.enter_context(tc.tile_pool(name="w", bufs=1))
        cpool = ctx.enter_context(tc.tile_pool(name="const", bufs=1))
        xpool = ctx.enter_context(tc.tile_pool(name="xT", bufs=1))
        rpool = ctx.enter_context(tc.tile_pool(name="rhs", bufs=2))
        epool = ctx.enter_context(tc.tile_pool(name="ev", bufs=3))
        xipool = ctx.enter_context(tc.tile_pool(name="xib", bufs=2))
        wnpool = ctx.enter_context(tc.tile_pool(name="win", bufs=2))
        tpool = ctx.enter_context(tc.tile_pool(name="tmp", bufs=2))
        spool = ctx.enter_context(tc.tile_pool(name="state", bufs=1))
        pspool = ctx.enter_context(tc.tile_pool(name="ps", bufs=2, space="PSUM"))
        fcpool = ctx.enter_context(tc.tile_pool(name="psfc", bufs=1, space="PSUM"))
        pgpool = ctx.enter_context(tc.tile_pool(name="pg", bufs=1, space="PSUM"))
        dpool = ctx.enter_context(tc.tile_pool(name="dram", bufs=1, space="DRAM"))
        hqpool = ctx.enter_context(tc.tile_pool(name="hseq", bufs=2, space="DRAM"))

        # persistent state
        h32 = spool.tile([128, NK, BC], F32, tag="h32")
        h16 = spool.tile([128, NK, BC], BF16, tag="h16")

        xT_sb = xpool.tile([128, nt], BF16, tag="xT")
        nc.sync.dma_start(xT_sb[:], xT[:])
        fcw_sb = cpool.tile([128, NK, PRED], BF16, tag="fcw")
        nc.sync.dma_start(fcw_sb[:], fcw[:])
        fcb_sb = cpool.tile([PRED, 1], F32, tag="fcb")
        nc.sync.dma_start(fcb_sb[:], fcb[:])
        biz_sb = []
        bhn_sb = []
        for l in range(NLAYERS):
            t = cpool.tile([128, NM], F32, tag=f"biz{l}")
            nc.sync.dma_start(t[:], bizs[l][:])
            biz_sb.append(t)
            t = cpool.tile([128, NK * BC], F32, tag=f"bhn{l}")
            nc.sync.dma_start(t[:], bhns[l][:])
            bhn_sb.append(t)

        # ---------------- layer 0 input projection (standalone) ----------------
        wih_sb = wpool.tile([128, NK, G], BF16, tag="wih")
        nc.sync.dma_start(wih_sb[:, 0:1, :], wihs[0][:])
        xi = dpool.tile([128, NM, nt], BF16, tag="xi0")

        def l0_inproj(noff, ncols):
            for m in range(NM):
                ps = pspool.tile([128, 512], F32, tag="ps")
                nc.tensor.matmul(ps[:, 0:ncols], wih_sb[:, 0, m * 128:(m + 1) * 128],
                                 xT_sb[:, bass.ds(noff, ncols)], start=True, stop=True)
                ev = epool.tile([128, 512], BF16, tag="ev")
                nc.scalar.activation(ev[:, 0:ncols], ps[:, 0:ncols], AF.Identity,
                                     bias=biz_sb[0][:, m:m + 1], scale=SCL)
                nc.sync.dma_start(xi[:, m, bass.ds(noff, ncols)], ev[:, 0:ncols])

        if nt >= 512:
            with tc.For_i(0, ntile) as n:
                l0_inproj(n * 512, 512)
        else:
            l0_inproj(0, nt)

        # ---------------- layers ----------------
        for l in range(NLAYERS):
            last = l == NLAYERS - 1
            whh_sb = wpool.tile([128, NK, G], BF16, tag="whh")
            nc.sync.dma_start(whh_sb[:], whhs[l][:])
            if not last:
                wih_sb = wpool.tile([128, NK, G], BF16, tag="wih")
                nc.sync.dma_start(wih_sb[:], wihs[l + 1][:])
                hseq = hqpool.tile([128, NK, nt], BF16, tag="hseq")
                xi_next = dpool.tile([128, NM, nt], BF16, tag=f"xi{(l + 1) % 2}")

            nc.vector.memset(h32[:], 0.0)
            nc.vector.memset(h16[:], 0.0)

            def inproj_unit(m, src_off, dst_off):
                """One m-chunk of layer l+1's inproj over one CB-col block."""
                psI = pspool.tile([128, CB], F32, tag="psI")
                for k in range(NK):
                    nc.tensor.matmul(psI[:], wih_sb[:, k, m * 128:(m + 1) * 128],
                                     rhsI[:, k, :], start=(k == 0), stop=(k == NK - 1))
                evI = epool.tile([128, CB], BF16, tag="evI")
                nc.scalar.activation(evI[:], psI[:], AF.Identity,
                                     bias=biz_sb[l + 1][:, m:m + 1], scale=SCL)
                nc.sync.dma_start(xi_next[:, m, bass.ds(dst_off, CB)], evI[:])

            def step(dt, xiba, xibb, winb, inp_off):
                # gate matmuls: z (m 8..15), r (m 0..7), n (m 16..23).
                # k-half-major order: all (m, k 0..3) then (m, k 4..7), so the
                # NEXT step's k 0..3 matmuls depend only on the first half of
                # h16 — the second half of the gate tail hides under them.
                pgz = pgpool.tile([128, NK, BC], F32, tag="pgz")
                pgr = pgpool.tile([128, NK, BC], F32, tag="pgr")
                pgn = pgpool.tile([128, NK, BC], F32, tag="pgn")
                for pg, mo in ((pgz, 8), (pgr, 0), (pgn, 16)):
                    for m in range(8):
                        for k in range(NK):
                            nc.tensor.matmul(
                                pg[:, m, :],
                                whh_sb[:, k, (m + mo) * 128:(m + mo + 1) * 128],
                                h16[:, k, :],
                                start=(k == 0), stop=(k == NK - 1))
                if dt < BLKA:
                    xs = xiba[:, :, dt * BC:(dt + 1) * BC]
                else:
                    xs = xibb[:, :, (dt - BLKA) * BC:(dt - BLKA + 1) * BC]
                # ---- z chain (overlaps r+n matmuls) ----
                zp = tpool.tile([128, 128], F32, tag="zp")
                nc.vector.tensor_add(zp[:], pgz[:], xs[:, 8:16, :])
                zt = tpool.tile([128, 128], F32, tag="zt")
                nc.scalar.activation(zt[:], zp[:], AF.Tanh, scale=0.5 / SCL)
                z = tpool.tile([128, 128], F32, tag="z")
                nc.vector.tensor_scalar(z[:], zt[:], 0.5, 0.5,
                                        mybir.AluOpType.mult,
                                        mybir.AluOpType.add)
                zc = tpool.tile([128, 128], F32, tag="zc")
                nc.vector.tensor_scalar(zc[:], zt[:], -0.5, 0.5,
                                        mybir.AluOpType.mult,
                                        mybir.AluOpType.add)
                zh = tpool.tile([128, 128], F32, tag="zh")
                nc.vector.tensor_mul(zh[:], z[:], h32[:])
                # ---- r chain (overlaps n matmuls) ----
                rp = tpool.tile([128, 128], F32, tag="rp")
                nc.vector.tensor_add(rp[:], pgr[:], xs[:, 0:8, :])
                rt = tpool.tile([128, 128], F32, tag="rt")
                nc.scalar.activation(rt[:], rp[:], AF.Tanh, scale=0.5 / SCL)
                r = tpool.tile([128, 128], F32, tag="r")
                nc.vector.tensor_scalar(r[:], rt[:], 0.5, 0.5,
                                        mybir.AluOpType.mult,
                                        mybir.AluOpType.add)
                # rb = r*b_hn + xs_n, precomputed off the critical tail
                rb1 = tpool.tile([128, 128], F32, tag="rb1")
                nc.vector.tensor_mul(rb1[:], r[:], bhn_sb[l][:])
                rb2 = tpool.tile([128, 128], F32, tag="rb2")
                nc.vector.tensor_add(rb2[:], rb1[:], xs[:, 16:24, :])
                # ---- n chain (the serial tail) ----
                t1 = tpool.tile([128, 128], F32, tag="t1")
                nc.vector.tensor_mul(t1[:], r[:], pgn[:])
                t2 = tpool.tile([128, 128], F32, tag="t2")
                nc.vector.tensor_add(t2[:], t1[:], rb2[:])
                nt_ = tpool.tile([128, 128], F32, tag="nt")
                nc.scalar.activation(nt_[:], t2[:], AF.Tanh, scale=1.0 / SCL)
                p1 = tpool.tile([128, 128], F32, tag="p1")
                nc.vector.tensor_mul(p1[:], zc[:], nt_[:])
                # h16 first: the next step's matmuls wait only on it
                nc.vector.tensor_add(h16[:], p1[:], zh[:])
                if winb is not None:
                    nc.vector.tensor_add(
                        winb[:, :, dt * BC:(dt + 1) * BC], p1[:], zh[:])
                nc.vector.tensor_add(h32[:], p1[:], zh[:])
                # interleaved next-layer inproj: fills the per-step tail
                # (tensor-queue: after this step's MMs, before the next step's)
                if inp_off is not None:
                    sm = (3 * dt) // 2
                    cnt = (3 * (dt + 1)) // 2 - sm
                    for m in range(sm, sm + cnt):
                        inproj_unit(m, inp_off, inp_off)

            def block(rec_off, inp_off):
                """Recurrence block at column offset rec_off; optionally the
                interleaved next-layer inproj over block at inp_off."""
                xiba = xipool.tile([128, NM, BLKA * BC], BF16, tag="xiba")
                nc.sync.dma_start(xiba[:], xi[:, :, bass.ds(rec_off, BLKA * BC)])
                xibb = xipool.tile([128, NM, (BLK - BLKA) * BC], BF16, tag="xibb")
                nc.sync.dma_start(xibb[:], xi[:, :, bass.ds(rec_off + BLKA * BC,
                                                            (BLK - BLKA) * BC)])
                winb = None
                if not last:
                    winb = wnpool.tile([128, NK, CB], BF16, tag="win")
                for dt in range(BLK):
                    step(dt, xiba, xibb, winb, inp_off)
                if not last:
                    nc.sync.dma_start(hseq[:, :, bass.ds(rec_off, CB)], winb[:])

            if last:
                if nblk > 1:
                    with tc.For_i(0, nblk) as blk:
                        block(blk * CB, None)
                else:
                    block(0, None)
            else:
                rhsI = None
                block(0, None)  # peeled: no previous block to project yet
                if nblk > 1:
                    with tc.For_i(0, nblk - 1) as j:
                        rhsI = rpool.tile([128, NK, CB], BF16, tag="rhsI")
                        nc.sync.dma_start(rhsI[:], hseq[:, :, bass.ds(j * CB, CB)])
                        block((j + 1) * CB, j * CB)
                # trailing inproj for the final block of this layer
                rhsI = rpool.tile([128, NK, CB], BF16, tag="rhsI")
                fin = (nblk - 1) * CB
                nc.sync.dma_start(rhsI[:], hseq[:, :, bass.ds(fin, CB)])
                for m in range(NM):
                    inproj_unit(m, fin, fin)
            if not last:
                xi = xi_next

        # ---------------- final FC ----------------
        psfc = fcpool.tile([PRED, BC], F32, tag="psfc")
        for k in range(NK):
            nc.tensor.matmul(psfc[:], fcw_sb[:, k, :], h16[:, k, :],
                             start=(k == 0), stop=(k == NK - 1))
        ofc = epool.tile([PRED, BC], F32, tag="ofc")
        nc.scalar.activation(ofc[:], psfc[:], AF.Identity, bias=fcb_sb[:])
        nc.sync.dma_start(out[:], ofc[:])

    nc.compile()
    return nc


def prep_inputs(inputs, T_=T):
    """Host-side layout preprocessing. Returns per-core input maps."""
    x = np.asarray(inputs["x"], np.float32)

    def chunkT(w):  # [G_out, K*128] -> [128, K, G_out] (lhsT layout)
        w = np.asarray(w, np.float32)
        gout, kin = w.shape
        return np.ascontiguousarray(
            w.T.reshape(kin // 128, 128, gout).transpose(1, 0, 2)
        ).astype(np_bf16)

    def chunkT_scl(w):  # [G_out, K*128] -> [128, K, G_out], pre-scaled by SCL
        w = np.asarray(w, np.float32) * SCL
        gout, kin = w.shape
        return np.ascontiguousarray(
            w.T.reshape(kin // 128, 128, gout).transpose(1, 0, 2)
        ).astype(np_bf16)

    shared = {}
    for l in range(NLAYERS):
        wih = np.asarray(inputs[f"w_ih_{l}"], np.float32)
        shared[f"wih{l}"] = chunkT(wih)
        shared[f"whh{l}"] = chunkT_scl(inputs[f"w_hh_{l}"])
        b_ih = np.asarray(inputs[f"b_ih_{l}"], np.float32)
        b_hh = np.asarray(inputs[f"b_hh_{l}"], np.float32)
        comb = b_ih.copy()
        comb[:2 * H] += b_hh[:2 * H]
        shared[f"biz{l}"] = np.ascontiguousarray(comb.reshape(NM, 128).T) * SCL
        bhn = b_hh[2 * H:].reshape(NK, 128).T  # [128, NK]
        shared[f"bhn{l}"] = np.ascontiguousarray(np.repeat(bhn, BC, axis=1)) * SCL
    shared["fcw"] = chunkT(inputs["fc_w"])
    shared["fcb"] = np.asarray(inputs["fc_b"], np.float32).reshape(PRED, 1)

    in_maps = []
    for c in range(NCORES):
        xc = x[c * BC:(c + 1) * BC, :T_, :]  # [BC, T, IN]
        xTc = np.ascontiguousarray(
            xc.transpose(2, 1, 0).reshape(IN, T_ * BC)
        ).astype(np_bf16)
        m = dict(shared)
        m["xT"] = xTc
        in_maps.append(m)
    return in_maps


_NC_CACHE = {}


def kernel(**inputs):
    if "nc" not in _NC_CACHE:
        _NC_CACHE["nc"] = build()
    nc = _NC_CACHE["nc"]
    in_maps = prep_inputs(inputs)
    res = run_bass_kernel_spmd(nc, in_maps, list(range(NCORES)))
    outs = []
    for c in range(NCORES):
        o = np.asarray(res.results[c]["out"], np.float32)  # [PRED, BC]
        outs.append(o.T)  # [BC, PRED]
    return np.concatenate(outs, axis=0)  # [B, PRED]


if __name__ == "__main__":
    rng = np.random.default_rng(0)
    k = 1.0 / np.sqrt(H)
    ins = {"x": rng.standard_normal((B, T, IN), dtype=np.float32)}
    for l in range(NLAYERS):
        ind = IN if l == 0 else H
        ins[f"w_ih_{l}"] = rng.uniform(-k, k, (G, ind)).astype(np.float32)
        ins[f"w_hh_{l}"] = rng.uniform(-k, k, (G, H)).astype(np.float32)
        ins[f"b_ih_{l}"] = rng.uniform(-k, k, (G,)).astype(np.float32)
        ins[f"b_hh_{l}"] = rng.uniform(-k, k, (G,)).astype(np.float32)
    ins["fc_w"] = rng.uniform(-k, k, (PRED, H)).astype(np.float32)
    ins["fc_b"] = rng.uniform(-k, k, (PRED,)).astype(np.float32)
    print(kernel(**ins).shape)


# revision 25
# speedup vs baseline: 1.2287x; 1.0126x over previous
"""Trainium2 Bass kernel for a 3-layer GRU (B=128, T=512, IN=128, H=1024, PRED=96).

Strategy: data-parallel over batch across 8 NeuronCores (B_core=16).

Layer l's recurrence (phase B) runs 512 sequential steps; per step 192
bf16 matmuls (K=128, M=128, N=16) accumulate W_hh @ h_t into three PSUM
tiles (z, r, n gate groups, issued in that order) so the z/r sigmoid
chains on DVE/ACT overlap the remaining matmul stream; only the n-gate
tail (r*pg_n -> +rb -> tanh -> combine) is serial per step.

Layer l+1's input projection is absorbed into layer l's recurrence
loop: after each step's matmuls, 1-2 m-chunks of W_ih_{l+1} @ hseq_l
(N=256 GEMMs over the previous block) are issued on the tensor queue.
They execute inside the per-step gate-math tail (otherwise PE-idle) and
keep the PE array busy enough that the HAM clock gate stays at 2.4 GHz.
Layer 0's input projection runs standalone up front (reads xT).

Weights are pre-scaled by SCL host-side; gates are computed at SCL*x
and descaled inside the tanh activations (free scale parameter).
Final FC done on-chip; host only transposes [96,16] -> [16,96] per core.
"""

import numpy as np
from contextlib import ExitStack

import concourse.bass as bass
import concourse.bacc as bacc
import concourse.mybir as mybir
import concourse.tile as tile
from concourse.bass_utils import run_bass_kernel_spmd

try:
    from ml_dtypes import bfloat16 as np_bf16
except ImportError:  # pragma: no cover
    import jax.numpy as jnp

    np_bf16 = jnp.bfloat16

B, T, IN, H, NLAYERS, PRED = 128, 512, 128, 1024, 3, 96
NCORES = 8
BC = B // NCORES  # 16
G = 3 * H  # 3072
NK = H // 128  # 8
NM = G // 128  # 24
BLK = 16  # recurrence steps per For_i iteration
BLKA = 4  # early sub-block of xi steps
CB = BLK * BC  # columns per block (256)
F32, BF16 = mybir.dt.float32, mybir.dt.bfloat16
SCL = 256.0  # weight pre-scale; descaled in the tanh activations
AF = mybir.ActivationFunctionType


def build(T_=T):
    nt = T_ * BC
    nblk = T_ // BLK
    ntile = nt // 512 if nt >= 512 else 1
    nc = bacc.Bacc("TRN2", target_bir_lowering=False, debug=False,
                   num_devices=NCORES)

    xT = nc.dram_tensor("xT", [128, nt], BF16, kind="ExternalInput")
    wihs = [
        nc.dram_tensor("wih0", [128, 1, G], BF16, kind="ExternalInput"),
        nc.dram_tensor("wih1", [128, NK, G], BF16, kind="ExternalInput"),
        nc.dram_tensor("wih2", [128, NK, G], BF16, kind="ExternalInput"),
    ]
    whhs = [nc.dram_tensor(f"whh{l}", [128, NK, G], BF16, kind="ExternalInput")
            for l in range(NLAYERS)]
    bizs = [nc.dram_tensor(f"biz{l}", [128, NM], F32, kind="ExternalInput")
            for l in range(NLAYERS)]
    bhns = [nc.dram_tensor(f"bhn{l}", [128, NK * BC], F32, kind="ExternalInput")
            for l in range(NLAYERS)]
    fcw = nc.dram_tensor("fcw", [128, NK, PRED], BF16, kind="ExternalInput")
    fcb = nc.dram_tensor("fcb", [PRED, 1], F32, kind="ExternalInput")
    out = nc.dram_tensor("out", [PRED, BC], F32, kind="ExternalOutput")

    with tile.TileContext(nc) as tc, ExitStack() as ctx:
        wpool = ctx.enter_context(tc.tile_pool(name="w", bufs=1))
        cpool = ctx.enter_context(tc.tile_pool(name="const", bufs=1))
        xpool = ctx.enter_context(tc.tile_pool(name="xT", bufs=1))
        rpool = ctx.enter_context(tc.tile_pool(name="rhs", bufs=2))
        epool = ctx.enter_context(tc.tile_pool(name="ev", bufs=3))
        xipool = ctx.enter_context(tc.tile_pool(name="xib", bufs=2))
        wnpool = ctx.enter_context(tc.tile_pool(name="win", bufs=2))
        tpool = ctx.enter_context(tc.tile_pool(name="tmp", bufs=2))
        spool = ctx.enter_context(tc.tile_pool(name="state", bufs=1))
        pspool = ctx.enter_context(tc.tile_pool(name="ps", bufs=2, space="PSUM"))
        fcpool = ctx.enter_context(tc.tile_pool(name="psfc", bufs=1, space="PSUM"))
        pgpool = ctx.enter_context(tc.tile_pool(name="pg", bufs=1, space="PSUM"))
        dpool = ctx.enter_context(tc.tile_pool(name="dram", bufs=1, space="DRAM"))
        hqpool = ctx.enter_context(tc.tile_pool(name="hseq", bufs=2, space="DRAM"))

        # persistent state
        h32 = spool.tile([128, NK, BC], F32, tag="h32")
        h16 = spool.tile([128, NK, BC], BF16, tag="h16")

        xT_sb = xpool.tile([128, nt], BF16, tag="xT")
        nc.sync.dma_start(xT_sb[:], xT[:])
        fcw_sb = cpool.tile([128, NK, PRED], BF16, tag="fcw")
        nc.sync.dma_start(fcw_sb[:], fcw[:])
        fcb_sb = cpool.tile([PRED, 1], F32, tag="fcb")
        nc.sync.dma_start(fcb_sb[:], fcb[:])
        biz_sb = []
        bhn_sb = []
        for l in range(NLAYERS):
            t = cpool.tile([128, NM], F32, tag=f"biz{l}")
            nc.sync.dma_start(t[:], bizs[l][:])
            biz_sb.append(t)
            t = cpool.tile([128, NK * BC], F32, tag=f"bhn{l}")
            nc.sync.dma_start(t[:], bhns[l][:])
            bhn_sb.append(t)

        # ---------------- layer 0 input projection (standalone) ----------------
        wih_sb = wpool.tile([128, NK, G], BF16, tag="wih")
        nc.sync.dma_start(wih_sb[:, 0:1, :], wihs[0][:])
        xi = dpool.tile([128, NM, nt], BF16, tag="xi0")

        def l0_inproj(noff, ncols):
            for m in range(NM):
                ps = pspool.tile([128, 512], F32, tag="ps")
                nc.tensor.matmul(ps[:, 0:ncols], wih_sb[:, 0, m * 128:(m + 1) * 128],
                                 xT_sb[:, bass.ds(noff, ncols)], start=True, stop=True)
                ev = epool.tile([128, 512], BF16, tag="ev")
                nc.scalar.activation(ev[:, 0:ncols], ps[:, 0:ncols], AF.Identity,
                                     bias=biz_sb[0][:, m:m + 1], scale=SCL)
                nc.sync.dma_start(xi[:, m, bass.ds(noff, ncols)], ev[:, 0:ncols])

        if nt >= 512:
            with tc.For_i(0, ntile) as n:
                l0_inproj(n * 512, 512)
        else:
            l0_inproj(0, nt)

        # ---------------- layers ----------------
        for l in range(NLAYERS):
            last = l == NLAYERS - 1
            whh_sb = wpool.tile([128, NK, G], BF16, tag="whh")
            nc.sync.dma_start(whh_sb[:], whhs[l][:])
            if not last:
                wih_sb = wpool.tile([128, NK, G], BF16, tag="wih")
                nc.sync.dma_start(wih_sb[:], wihs[l + 1][:])
                hseq = hqpool.tile([128, NK, nt], BF16, tag="hseq")
                xi_next = dpool.tile([128, NM, nt], BF16, tag=f"xi{(l + 1) % 2}")

            nc.vector.memset(h32[:], 0.0)
            nc.vector.memset(h16[:], 0.0)

            def inproj_unit(m, src_off, dst_off):
                """One m-chunk of layer l+1's inproj over one CB-col block."""
                psI = pspool.tile([128, CB], F32, tag="psI")
                for k in range(NK):
                    nc.tensor.matmul(psI[:], wih_sb[:, k, m * 128:(m + 1) * 128],
                                     rhsI[:, k, :], start=(k == 0), stop=(k == NK - 1))
                evI = epool.tile([128, CB], BF16, tag="evI")
                nc.scalar.activation(evI[:], psI[:], AF.Identity,
                                     bias=biz_sb[l + 1][:, m:m + 1], scale=SCL)
                nc.sync.dma_start(xi_next[:, m, bass.ds(dst_off, CB)], evI[:])

            def step(dt, xiba, xibb, winb, inp_off):
                # gate matmuls: z (m 8..15), r (m 0..7), n (m 16..23).
                # k-half-major order: all (m, k 0..3) then (m, k 4..7), so the
                # NEXT step's k 0..3 matmuls depend only on the first half of
                # h16 — the second half of the gate tail hides under them.
                pgz = pgpool.tile([128, NK, BC], F32, tag="pgz")
                pgr = pgpool.tile([128, NK, BC], F32, tag="pgr")
                pgn = pgpool.tile([128, NK, BC], F32, tag="pgn")
                for pg, mo in ((pgz, 8), (pgr, 0), (pgn, 16)):
                    for m in range(8):
                        for k in range(NK):
                            nc.tensor.matmul(
                                pg[:, m, :],
                                whh_sb[:, k, (m + mo) * 128:(m + mo + 1) * 128],
                                h16[:, k, :],
                                start=(k == 0), stop=(k == NK - 1))
                if dt < BLKA:
                    xs = xiba[:, :, dt * BC:(dt + 1) * BC]
                else:
                    xs = xibb[:, :, (dt - BLKA) * BC:(dt - BLKA + 1) * BC]
                # ---- z chain (overlaps r+n matmuls) ----
                zp = tpool.tile([128, 128], F32, tag="zp")
                nc.vector.tensor_add(zp[:], pgz[:], xs[:, 8:16, :])
                zt = tpool.tile([128, 128], F32, tag="zt")
                nc.scalar.activation(zt[:], zp[:], AF.Tanh, scale=0.5 / SCL)
                z = tpool.tile([128, 128], F32, tag="z")
                nc.vector.tensor_scalar(z[:], zt[:], 0.5, 0.5,
                                        mybir.AluOpType.mult,
                                        mybir.AluOpType.add)
                zc = tpool.tile([128, 128], F32, tag="zc")
                nc.vector.tensor_scalar(zc[:], zt[:], -0.5, 0.5,
                                        mybir.AluOpType.mult,
                                        mybir.AluOpType.add)
                zh = tpool.tile([128, 128], F32, tag="zh")
                nc.vector.tensor_mul(zh[:], z[:], h32[:])
                # ---- r chain (overlaps n matmuls) ----
                rp = tpool.tile([128, 128], F32, tag="rp")
                nc.vector.tensor_add(rp[:], pgr[:], xs[:, 0:8, :])
                rt = tpool.tile([128, 128], F32, tag="rt")
                nc.scalar.activation(rt[:], rp[:], AF.Tanh, scale=0.5 / SCL)
                r = tpool.tile([128, 128], F32, tag="r")
                nc.vector.tensor_scalar(r[:], rt[:], 0.5, 0.5,
                                        mybir.AluOpType.mult,
                                        mybir.AluOpType.add)
                # rb = r*b_hn + xs_n, precomputed off the critical tail
                rb1 = tpool.tile([128, 128], F32, tag="rb1")
                nc.vector.tensor_mul(rb1[:], r[:], bhn_sb[l][:])
                rb2 = tpool.tile([128, 128], F32, tag="rb2")
                nc.vector.tensor_add(rb2[:], rb1[:], xs[:, 16:24, :])
                # ---- n chain (the serial tail) ----
                t1 = tpool.tile([128, 128], F32, tag="t1")
                nc.vector.tensor_mul(t1[:], r[:], pgn[:])
                t2 = tpool.tile([128, 128], F32, tag="t2")
                nc.vector.tensor_add(t2[:], t1[:], rb2[:])
                nt_ = tpool.tile([128, 128], F32, tag="nt")
                nc.scalar.activation(nt_[:], t2[:], AF.Tanh, scale=1.0 / SCL)
                p1 = tpool.tile([128, 128], F32, tag="p1")
                nc.vector.tensor_mul(p1[:], zc[:], nt_[:])
                # h16 first: the next step's matmuls wait only on it
                nc.vector.tensor_add(h16[:], p1[:], zh[:])
                if winb is not None:
                    nc.vector.tensor_add(
                        winb[:, :, dt * BC:(dt + 1) * BC], p1[:], zh[:])
                nc.vector.tensor_add(h32[:], p1[:], zh[:])
                # interleaved next-layer inproj: fills the per-step tail
                # (tensor-queue: after this step's MMs, before the next step's)
                if inp_off is not None:
                    sm = (3 * dt) // 2
                    cnt = (3 * (dt + 1)) // 2 - sm
                    for m in range(sm, sm + cnt):
                        inproj_unit(m, inp_off, inp_off)

            def block(rec_off, inp_off):
                """Recurrence block at column offset rec_off; optionally the
                interleaved next-layer inproj over block at inp_off."""
                xiba = xipool.tile([128, NM, BLKA * BC], BF16, tag="xiba")
                nc.sync.dma_start(xiba[:], xi[:, :, bass.ds(rec_off, BLKA * BC)])
                xibb = xipool.tile([128, NM, (BLK - BLKA) * BC], BF16, tag="xibb")
                nc.sync.dma_start(xibb[:], xi[:, :, bass.ds(rec_off + BLKA * BC,
                                                            (BLK - BLKA) * BC)])
                winb = None
                if not last:
                    winb = wnpool.tile([128, NK, CB], BF16, tag="win")
                for dt in range(BLK):
                    step(dt, xiba, xibb, winb, inp_off)
                if not last:
                    nc.sync.dma_start(hseq[:, :, bass.ds(rec_off, CB)], winb[:])

            # 2 blocks per For_i iteration: halves the ~10us per-iteration
            # cross-engine rendezvous bubble at hardware-loop boundaries
            if last:
                if nblk > 1:
                    with tc.For_i(0, nblk // 2) as blk:
                        block(blk * (2 * CB), None)
                        block(blk * (2 * CB) + CB, None)
                else:
                    block(0, None)
            else:
                rhsI = None
                block(0, None)  # peeled: no previous block to project yet
                if nblk > 1:
                    # blocks 1..nblk-1 with inproj lagging one block
                    with tc.For_i(0, (nblk - 2) // 2) as j:
                        rhsI = rpool.tile([128, NK, CB], BF16, tag="rhsI")
                        nc.sync.dma_start(rhsI[:],
                                          hseq[:, :, bass.ds(j * (2 * CB), CB)])
                        block(j * (2 * CB) + CB, j * (2 * CB))
                        rhsI = rpool.tile([128, NK, CB], BF16, tag="rhsI")
                        nc.sync.dma_start(rhsI[:],
                                          hseq[:, :, bass.ds(j * (2 * CB) + CB, CB)])
                        block(j * (2 * CB) + 2 * CB, j * (2 * CB) + CB)
                    # peeled final block (odd count): recurrence + inproj of prev
                    rhsI = rpool.tile([128, NK, CB], BF16, tag="rhsI")
                    nc.sync.dma_start(rhsI[:], hseq[:, :, bass.ds((nblk - 2) * CB, CB)])
                    block((nblk - 1) * CB, (nblk - 2) * CB)
                # trailing inproj for the final block of this layer
                rhsI = rpool.tile([128, NK, CB], BF16, tag="rhsI")
                fin = (nblk - 1) * CB
                nc.sync.dma_start(rhsI[:], hseq[:, :, bass.ds(fin, CB)])
                for m in range(NM):
                    inproj_unit(m, fin, fin)
            if not last:
                xi = xi_next

        # ---------------- final FC ----------------
        psfc = fcpool.tile([PRED, BC], F32, tag="psfc")
        for k in range(NK):
            nc.tensor.matmul(psfc[:], fcw_sb[:, k, :], h16[:, k, :],
                             start=(k == 0), stop=(k == NK - 1))
        ofc = epool.tile([PRED, BC], F32, tag="ofc")
        nc.scalar.activation(ofc[:], psfc[:], AF.Identity, bias=fcb_sb[:])
        nc.sync.dma_start(out[:], ofc[:])

    nc.compile()
    return nc


def prep_inputs(inputs, T_=T):
    """Host-side layout preprocessing. Returns per-core input maps."""
    x = np.asarray(inputs["x"], np.float32)

    def chunkT(w):  # [G_out, K*128] -> [128, K, G_out] (lhsT layout)
        w = np.asarray(w, np.float32)
        gout, kin = w.shape
        return np.ascontiguousarray(
            w.T.reshape(kin // 128, 128, gout).transpose(1, 0, 2)
        ).astype(np_bf16)

    def chunkT_scl(w):  # [G_out, K*128] -> [128, K, G_out], pre-scaled by SCL
        w = np.asarray(w, np.float32) * SCL
        gout, kin = w.shape
        return np.ascontiguousarray(
            w.T.reshape(kin // 128, 128, gout).transpose(1, 0, 2)
        ).astype(np_bf16)

    shared = {}
    for l in range(NLAYERS):
        wih = np.asarray(inputs[f"w_ih_{l}"], np.float32)
        shared[f"wih{l}"] = chunkT(wih)
        shared[f"whh{l}"] = chunkT_scl(inputs[f"w_hh_{l}"])
        b_ih = np.asarray(inputs[f"b_ih_{l}"], np.float32)
        b_hh = np.asarray(inputs[f"b_hh_{l}"], np.float32)
        comb = b_ih.copy()
        comb[:2 * H] += b_hh[:2 * H]
        shared[f"biz{l}"] = np.ascontiguousarray(comb.reshape(NM, 128).T) * SCL
        bhn = b_hh[2 * H:].reshape(NK, 128).T  # [128, NK]
        shared[f"bhn{l}"] = np.ascontiguousarray(np.repeat(bhn, BC, axis=1)) * SCL
    shared["fcw"] = chunkT(inputs["fc_w"])
    shared["fcb"] = np.asarray(inputs["fc_b"], np.float32).reshape(PRED, 1)

    in_maps = []
    for c in range(NCORES):
        xc = x[c * BC:(c + 1) * BC, :T_, :]  # [BC, T, IN]
        xTc = np.ascontiguousarray(
            xc.transpose(2, 1, 0).reshape(IN, T_ * BC)
        ).astype(np_bf16)
        m = dict(shared)
        m["xT"] = xTc
        in_maps.append(m)
    return in_maps


_NC_CACHE = {}


def kernel(**inputs):
    if "nc" not in _NC_CACHE:
        _NC_CACHE["nc"] = build()
    nc = _NC_CACHE["nc"]
    in_maps = prep_inputs(inputs)
    res = run_bass_kernel_spmd(nc, in_maps, list(range(NCORES)))
    outs = []
    for c in range(NCORES):
        o = np.asarray(res.results[c]["out"], np.float32)  # [PRED, BC]
        outs.append(o.T)  # [BC, PRED]
    return np.concatenate(outs, axis=0)  # [B, PRED]


if __name__ == "__main__":
    rng = np.random.default_rng(0)
    k = 1.0 / np.sqrt(H)
    ins = {"x": rng.standard_normal((B, T, IN), dtype=np.float32)}
    for l in range(NLAYERS):
        ind = IN if l == 0 else H
        ins[f"w_ih_{l}"] = rng.uniform(-k, k, (G, ind)).astype(np.float32)
        ins[f"w_hh_{l}"] = rng.uniform(-k, k, (G, H)).astype(np.float32)
        ins[f"b_ih_{l}"] = rng.uniform(-k, k, (G,)).astype(np.float32)
        ins[f"b_hh_{l}"] = rng.uniform(-k, k, (G,)).astype(np.float32)
    ins["fc_w"] = rng.uniform(-k, k, (PRED, H)).astype(np.float32)
    ins["fc_b"] = rng.uniform(-k, k, (PRED,)).astype(np.float32)
    print(kernel(**ins).shape)


# revision 26
# speedup vs baseline: 1.2555x; 1.0218x over previous
"""Trainium2 Bass kernel for a 3-layer GRU (B=128, T=512, IN=128, H=1024, PRED=96).

Strategy: data-parallel over batch across 8 NeuronCores (B_core=16).

Layer l's recurrence (phase B) runs 512 sequential steps; per step 192
bf16 matmuls (K=128, M=128, N=16) accumulate W_hh @ h_t into three PSUM
tiles (z, r, n gate groups, issued in that order) so the z/r sigmoid
chains on DVE/ACT overlap the remaining matmul stream; only the n-gate
tail (r*pg_n -> +rb -> tanh -> combine) is serial per step.

Layer l+1's input projection is absorbed into layer l's recurrence
loop: after each step's matmuls, 1-2 m-chunks of W_ih_{l+1} @ hseq_l
(N=256 GEMMs over the previous block) are issued on the tensor queue.
They execute inside the per-step gate-math tail (otherwise PE-idle) and
keep the PE array busy enough that the HAM clock gate stays at 2.4 GHz.
Layer 0's input projection runs standalone up front (reads xT).

Weights are pre-scaled by SCL host-side; gates are computed at SCL*x
and descaled inside the tanh activations (free scale parameter).
Final FC done on-chip; host only transposes [96,16] -> [16,96] per core.
"""

import numpy as np
from contextlib import ExitStack

import concourse.bass as bass
import concourse.bacc as bacc
import concourse.mybir as mybir
import concourse.tile as tile
from concourse.bass_utils import run_bass_kernel_spmd

try:
    from ml_dtypes import bfloat16 as np_bf16
except ImportError:  # pragma: no cover
    import jax.numpy as jnp

    np_bf16 = jnp.bfloat16

B, T, IN, H, NLAYERS, PRED = 128, 512, 128, 1024, 3, 96
NCORES = 8
BC = B // NCORES  # 16
G = 3 * H  # 3072
NK = H // 128  # 8
NM = G // 128  # 24
BLK = 16  # recurrence steps per For_i iteration
BLKA = 4  # early sub-block of xi steps
CB = BLK * BC  # columns per block (256)
F32, BF16 = mybir.dt.float32, mybir.dt.bfloat16
SCL = 256.0  # weight pre-scale; descaled in the tanh activations
AF = mybir.ActivationFunctionType


def build(T_=T):
    nt = T_ * BC
    nblk = T_ // BLK
    ntile = nt // 512 if nt >= 512 else 1
    nc = bacc.Bacc("TRN2", target_bir_lowering=False, debug=False,
                   num_devices=NCORES)

    xT = nc.dram_tensor("xT", [128, nt], BF16, kind="ExternalInput")
    wihs = [
        nc.dram_tensor("wih0", [128, 1, G], BF16, kind="ExternalInput"),
        nc.dram_tensor("wih1", [128, NK, G], BF16, kind="ExternalInput"),
        nc.dram_tensor("wih2", [128, NK, G], BF16, kind="ExternalInput"),
    ]
    whhs = [nc.dram_tensor(f"whh{l}", [128, NK, G], BF16, kind="ExternalInput")
            for l in range(NLAYERS)]
    bizs = [nc.dram_tensor(f"biz{l}", [128, NM], F32, kind="ExternalInput")
            for l in range(NLAYERS)]
    bhns = [nc.dram_tensor(f"bhn{l}", [128, NK * BC], F32, kind="ExternalInput")
            for l in range(NLAYERS)]
    fcw = nc.dram_tensor("fcw", [128, NK, PRED], BF16, kind="ExternalInput")
    fcb = nc.dram_tensor("fcb", [PRED, 1], F32, kind="ExternalInput")
    out = nc.dram_tensor("out", [PRED, BC], F32, kind="ExternalOutput")

    with tile.TileContext(nc) as tc, ExitStack() as ctx:
        wpool = ctx.enter_context(tc.tile_pool(name="w", bufs=1))
        cpool = ctx.enter_context(tc.tile_pool(name="const", bufs=1))
        xpool = ctx.enter_context(tc.tile_pool(name="xT", bufs=1))
        rpool = ctx.enter_context(tc.tile_pool(name="rhs", bufs=2))
        epool = ctx.enter_context(tc.tile_pool(name="ev", bufs=3))
        xipool = ctx.enter_context(tc.tile_pool(name="xib", bufs=2))
        wnpool = ctx.enter_context(tc.tile_pool(name="win", bufs=2))
        tpool = ctx.enter_context(tc.tile_pool(name="tmp", bufs=2))
        spool = ctx.enter_context(tc.tile_pool(name="state", bufs=1))
        pspool = ctx.enter_context(tc.tile_pool(name="ps", bufs=2, space="PSUM"))
        fcpool = ctx.enter_context(tc.tile_pool(name="psfc", bufs=1, space="PSUM"))
        pgpool = ctx.enter_context(tc.tile_pool(name="pg", bufs=1, space="PSUM"))
        dpool = ctx.enter_context(tc.tile_pool(name="dram", bufs=1, space="DRAM"))
        hqpool = ctx.enter_context(tc.tile_pool(name="hseq", bufs=2, space="DRAM"))

        # persistent state
        h32 = spool.tile([128, NK, BC], F32, tag="h32")
        h16 = spool.tile([128, NK, BC], BF16, tag="h16")

        xT_sb = xpool.tile([128, nt], BF16, tag="xT")
        nc.sync.dma_start(xT_sb[:], xT[:])
        fcw_sb = cpool.tile([128, NK, PRED], BF16, tag="fcw")
        nc.sync.dma_start(fcw_sb[:], fcw[:])
        fcb_sb = cpool.tile([PRED, 1], F32, tag="fcb")
        nc.sync.dma_start(fcb_sb[:], fcb[:])
        biz_sb = []
        bhn_sb = []
        for l in range(NLAYERS):
            t = cpool.tile([128, NM], F32, tag=f"biz{l}")
            nc.sync.dma_start(t[:], bizs[l][:])
            biz_sb.append(t)
            t = cpool.tile([128, NK * BC], F32, tag=f"bhn{l}")
            nc.sync.dma_start(t[:], bhns[l][:])
            bhn_sb.append(t)

        # ---------------- layer 0 input projection (standalone) ----------------
        wih_sb = wpool.tile([128, NK, G], BF16, tag="wih")
        nc.sync.dma_start(wih_sb[:, 0:1, :], wihs[0][:])
        xi = dpool.tile([128, NM, nt], BF16, tag="xi0")

        def l0_inproj(noff, ncols):
            for m in range(NM):
                ps = pspool.tile([128, 512], F32, tag="ps")
                nc.tensor.matmul(ps[:, 0:ncols], wih_sb[:, 0, m * 128:(m + 1) * 128],
                                 xT_sb[:, bass.ds(noff, ncols)], start=True, stop=True)
                ev = epool.tile([128, 512], BF16, tag="ev")
                nc.scalar.activation(ev[:, 0:ncols], ps[:, 0:ncols], AF.Identity,
                                     bias=biz_sb[0][:, m:m + 1], scale=SCL)
                nc.sync.dma_start(xi[:, m, bass.ds(noff, ncols)], ev[:, 0:ncols])

        if nt >= 512:
            with tc.For_i(0, ntile) as n:
                l0_inproj(n * 512, 512)
        else:
            l0_inproj(0, nt)

        # ---------------- layers ----------------
        for l in range(NLAYERS):
            last = l == NLAYERS - 1
            whh_sb = wpool.tile([128, NK, G], BF16, tag="whh")
            nc.sync.dma_start(whh_sb[:], whhs[l][:])
            if not last:
                wih_sb = wpool.tile([128, NK, G], BF16, tag="wih")
                nc.sync.dma_start(wih_sb[:], wihs[l + 1][:])
                hseq = hqpool.tile([128, NK, nt], BF16, tag="hseq")
                xi_next = dpool.tile([128, NM, nt], BF16, tag=f"xi{(l + 1) % 2}")

            nc.vector.memset(h32[:], 0.0)
            nc.vector.memset(h16[:], 0.0)

            def inproj_unit(m, src_off, dst_off):
                """One m-chunk of layer l+1's inproj over one CB-col block."""
                psI = pspool.tile([128, CB], F32, tag="psI")
                for k in range(NK):
                    nc.tensor.matmul(psI[:], wih_sb[:, k, m * 128:(m + 1) * 128],
                                     rhsI[:, k, :], start=(k == 0), stop=(k == NK - 1))
                evI = epool.tile([128, CB], BF16, tag="evI")
                nc.scalar.activation(evI[:], psI[:], AF.Identity,
                                     bias=biz_sb[l + 1][:, m:m + 1], scale=SCL)
                nc.sync.dma_start(xi_next[:, m, bass.ds(dst_off, CB)], evI[:])

            def step(dt, xiba, xibb, winb, inp_off):
                # gate matmuls: z (m 8..15), r (m 0..7), n (m 16..23).
                # k-half-major order: all (m, k 0..3) then (m, k 4..7), so the
                # NEXT step's k 0..3 matmuls depend only on the first half of
                # h16 — the second half of the gate tail hides under them.
                pgz = pgpool.tile([128, NK, BC], F32, tag="pgz")
                pgr = pgpool.tile([128, NK, BC], F32, tag="pgr")
                pgn = pgpool.tile([128, NK, BC], F32, tag="pgn")
                for pg, mo in ((pgz, 8), (pgr, 0), (pgn, 16)):
                    for m in range(8):
                        for k in range(NK):
                            nc.tensor.matmul(
                                pg[:, m, :],
                                whh_sb[:, k, (m + mo) * 128:(m + mo + 1) * 128],
                                h16[:, k, :],
                                start=(k == 0), stop=(k == NK - 1))
                if dt < BLKA:
                    xs = xiba[:, :, dt * BC:(dt + 1) * BC]
                else:
                    xs = xibb[:, :, (dt - BLKA) * BC:(dt - BLKA + 1) * BC]
                # ---- z chain (overlaps r+n matmuls) ----
                zp = tpool.tile([128, 128], F32, tag="zp")
                nc.vector.tensor_add(zp[:], pgz[:], xs[:, 8:16, :])
                zt = tpool.tile([128, 128], F32, tag="zt")
                nc.scalar.activation(zt[:], zp[:], AF.Tanh, scale=0.5 / SCL)
                z = tpool.tile([128, 128], F32, tag="z")
                nc.vector.tensor_scalar(z[:], zt[:], 0.5, 0.5,
                                        mybir.AluOpType.mult,
                                        mybir.AluOpType.add)
                zc = tpool.tile([128, 128], F32, tag="zc")
                nc.vector.tensor_scalar(zc[:], zt[:], -0.5, 0.5,
                                        mybir.AluOpType.mult,
                                        mybir.AluOpType.add)
                zh = tpool.tile([128, 128], F32, tag="zh")
                nc.vector.tensor_mul(zh[:], z[:], h32[:])
                # ---- r chain (overlaps n matmuls) ----
                rp = tpool.tile([128, 128], F32, tag="rp")
                nc.vector.tensor_add(rp[:], pgr[:], xs[:, 0:8, :])
                rt = tpool.tile([128, 128], F32, tag="rt")
                nc.scalar.activation(rt[:], rp[:], AF.Tanh, scale=0.5 / SCL)
                r = tpool.tile([128, 128], F32, tag="r")
                nc.vector.tensor_scalar(r[:], rt[:], 0.5, 0.5,
                                        mybir.AluOpType.mult,
                                        mybir.AluOpType.add)
                # rb = r*b_hn + xs_n, precomputed off the critical tail
                rb1 = tpool.tile([128, 128], F32, tag="rb1")
                nc.vector.tensor_mul(rb1[:], r[:], bhn_sb[l][:])
                rb2 = tpool.tile([128, 128], F32, tag="rb2")
                nc.vector.tensor_add(rb2[:], rb1[:], xs[:, 16:24, :])
                # ---- n chain (the serial tail), two H-halves ----
                # matmul issue order is unchanged (k-inner); only the tail ops
                # are halved so h16[:, 0:4] lands early and the next step's
                # k 0..3 matmuls start while the second half still computes.
                p1h = {}
                for lo in (0, 64):
                    hs = slice(lo, lo + 64)
                    ks = slice(lo // 16, lo // 16 + 4)
                    t1 = tpool.tile([128, 64], F32, tag=f"t1{lo}")
                    nc.vector.tensor_mul(t1[:], r[:, hs], pgn[:, ks, :])
                    t2 = tpool.tile([128, 64], F32, tag=f"t2{lo}")
                    nc.vector.tensor_add(t2[:], t1[:], rb2[:, hs])
                    nt_ = tpool.tile([128, 64], F32, tag=f"nt{lo}")
                    nc.scalar.activation(nt_[:], t2[:], AF.Tanh, scale=1.0 / SCL)
                    p1 = tpool.tile([128, 64], F32, tag=f"p1{lo}")
                    nc.vector.tensor_mul(p1[:], zc[:, hs], nt_[:])
                    nc.vector.tensor_add(h16[:, ks, :], p1[:], zh[:, hs])
                    p1h[lo] = p1
                # off-critical writes after both h16 halves
                for lo in (0, 64):
                    hs = slice(lo, lo + 64)
                    ks = slice(lo // 16, lo // 16 + 4)
                    if winb is not None:
                        nc.vector.tensor_add(
                            winb[:, ks, dt * BC:(dt + 1) * BC], p1h[lo][:], zh[:, hs])
                    nc.vector.tensor_add(h32[:, ks, :], p1h[lo][:], zh[:, hs])
                # interleaved next-layer inproj: fills the per-step tail
                # (tensor-queue: after this step's MMs, before the next step's)
                if inp_off is not None:
                    sm = (3 * dt) // 2
                    cnt = (3 * (dt + 1)) // 2 - sm
                    for m in range(sm, sm + cnt):
                        inproj_unit(m, inp_off, inp_off)

            def block(rec_off, inp_off):
                """Recurrence block at column offset rec_off; optionally the
                interleaved next-layer inproj over block at inp_off."""
                xiba = xipool.tile([128, NM, BLKA * BC], BF16, tag="xiba")
                nc.sync.dma_start(xiba[:], xi[:, :, bass.ds(rec_off, BLKA * BC)])
                xibb = xipool.tile([128, NM, (BLK - BLKA) * BC], BF16, tag="xibb")
                nc.sync.dma_start(xibb[:], xi[:, :, bass.ds(rec_off + BLKA * BC,
                                                            (BLK - BLKA) * BC)])
                winb = None
                if not last:
                    winb = wnpool.tile([128, NK, CB], BF16, tag="win")
                for dt in range(BLK):
                    step(dt, xiba, xibb, winb, inp_off)
                if not last:
                    nc.sync.dma_start(hseq[:, :, bass.ds(rec_off, CB)], winb[:])

            # 2 blocks per For_i iteration: halves the ~10us per-iteration
            # cross-engine rendezvous bubble at hardware-loop boundaries
            if last:
                if nblk > 1:
                    with tc.For_i(0, nblk // 2) as blk:
                        block(blk * (2 * CB), None)
                        block(blk * (2 * CB) + CB, None)
                else:
                    block(0, None)
            else:
                rhsI = None
                block(0, None)  # peeled: no previous block to project yet
                if nblk > 1:
                    # blocks 1..nblk-1 with inproj lagging one block
                    with tc.For_i(0, (nblk - 2) // 2) as j:
                        rhsI = rpool.tile([128, NK, CB], BF16, tag="rhsI")
                        nc.sync.dma_start(rhsI[:],
                                          hseq[:, :, bass.ds(j * (2 * CB), CB)])
                        block(j * (2 * CB) + CB, j * (2 * CB))
                        rhsI = rpool.tile([128, NK, CB], BF16, tag="rhsI")
                        nc.sync.dma_start(rhsI[:],
                                          hseq[:, :, bass.ds(j * (2 * CB) + CB, CB)])
                        block(j * (2 * CB) + 2 * CB, j * (2 * CB) + CB)
                    # peeled final block (odd count): recurrence + inproj of prev
                    rhsI = rpool.tile([128, NK, CB], BF16, tag="rhsI")
                    nc.sync.dma_start(rhsI[:], hseq[:, :, bass.ds((nblk - 2) * CB, CB)])
                    block((nblk - 1) * CB, (nblk - 2) * CB)
                # trailing inproj for the final block of this layer
                rhsI = rpool.tile([128, NK, CB], BF16, tag="rhsI")
                fin = (nblk - 1) * CB
                nc.sync.dma_start(rhsI[:], hseq[:, :, bass.ds(fin, CB)])
                for m in range(NM):
                    inproj_unit(m, fin, fin)
            if not last:
                xi = xi_next

        # ---------------- final FC ----------------
        psfc = fcpool.tile([PRED, BC], F32, tag="psfc")
        for k in range(NK):
            nc.tensor.matmul(psfc[:], fcw_sb[:, k, :], h16[:, k, :],
                             start=(k == 0), stop=(k == NK - 1))
        ofc = epool.tile([PRED, BC], F32, tag="ofc")
        nc.scalar.activation(ofc[:], psfc[:], AF.Identity, bias=fcb_sb[:])
        nc.sync.dma_start(out[:], ofc[:])

    nc.compile()
    return nc


def prep_inputs(inputs, T_=T):
    """Host-side layout preprocessing. Returns per-core input maps."""
    x = np.asarray(inputs["x"], np.float32)

    def chunkT(w):  # [G_out, K*128] -> [128, K, G_out] (lhsT layout)
        w = np.asarray(w, np.float32)
        gout, kin = w.shape
        return np.ascontiguousarray(
            w.T.reshape(kin // 128, 128, gout).transpose(1, 0, 2)
        ).astype(np_bf16)

    def chunkT_scl(w):  # [G_out, K*128] -> [128, K, G_out], pre-scaled by SCL
        w = np.asarray(w, np.float32) * SCL
        gout, kin = w.shape
        return np.ascontiguousarray(
            w.T.reshape(kin // 128, 128, gout).transpose(1, 0, 2)
        ).astype(np_bf16)

    shared = {}
    for l in range(NLAYERS):
        wih = np.asarray(inputs[f"w_ih_{l}"], np.float32)
        shared[f"wih{l}"] = chunkT(wih)
        shared[f"whh{l}"] = chunkT_scl(inputs[f"w_hh_{l}"])
        b_ih = np.asarray(inputs[f"b_ih_{l}"], np.float32)
        b_hh = np.asarray(inputs[f"b_hh_{l}"], np.float32)
        comb = b_ih.copy()
        comb[:2 * H] += b_hh[:2 * H]
        shared[f"biz{l}"] = np.ascontiguousarray(comb.reshape(NM, 128).T) * SCL
        bhn = b_hh[2 * H:].reshape(NK, 128).T  # [128, NK]
        shared[f"bhn{l}"] = np.ascontiguousarray(np.repeat(bhn, BC, axis=1)) * SCL
    shared["fcw"] = chunkT(inputs["fc_w"])
    shared["fcb"] = np.asarray(inputs["fc_b"], np.float32).reshape(PRED, 1)

    in_maps = []
    for c in range(NCORES):
        xc = x[c * BC:(c + 1) * BC, :T_, :]  # [BC, T, IN]
        xTc = np.ascontiguousarray(
            xc.transpose(2, 1, 0).reshape(IN, T_ * BC)
        ).astype(np_bf16)
        m = dict(shared)
        m["xT"] = xTc
        in_maps.append(m)
    return in_maps


_NC_CACHE = {}


def kernel(**inputs):
    if "nc" not in _NC_CACHE:
        _NC_CACHE["nc"] = build()
    nc = _NC_CACHE["nc"]
    in_maps = prep_inputs(inputs)
    res = run_bass_kernel_spmd(nc, in_maps, list(range(NCORES)))
    outs = []
    for c in range(NCORES):
        o = np.asarray(res.results[c]["out"], np.float32)  # [PRED, BC]
        outs.append(o.T)  # [BC, PRED]
    return np.concatenate(outs, axis=0)  # [B, PRED]


if __name__ == "__main__":
    rng = np.random.default_rng(0)
    k = 1.0 / np.sqrt(H)
    ins = {"x": rng.standard_normal((B, T, IN), dtype=np.float32)}
    for l in range(NLAYERS):
        ind = IN if l == 0 else H
        ins[f"w_ih_{l}"] = rng.uniform(-k, k, (G, ind)).astype(np.float32)
        ins[f"w_hh_{l}"] = rng.uniform(-k, k, (G, H)).astype(np.float32)
        ins[f"b_ih_{l}"] = rng.uniform(-k, k, (G,)).astype(np.float32)
        ins[f"b_hh_{l}"] = rng.uniform(-k, k, (G,)).astype(np.float32)
    ins["fc_w"] = rng.uniform(-k, k, (PRED, H)).astype(np.float32)
    ins["fc_b"] = rng.uniform(-k, k, (PRED,)).astype(np.float32)
    print(kernel(**ins).shape)


# revision 30
# speedup vs baseline: 1.2590x; 1.0028x over previous
"""Trainium2 Bass kernel for a 3-layer GRU (B=128, T=512, IN=128, H=1024, PRED=96).

Strategy: data-parallel over batch across 8 NeuronCores (B_core=16).

Layer l's recurrence (phase B) runs 512 sequential steps; per step 192
bf16 matmuls (K=128, M=128, N=16) accumulate W_hh @ h_t into three PSUM
tiles (z, r, n gate groups, issued in that order) so the z/r sigmoid
chains on DVE/ACT overlap the remaining matmul stream; only the n-gate
tail (r*pg_n -> +rb -> tanh -> combine) is serial per step.

Layer l+1's input projection is absorbed into layer l's recurrence
loop: after each step's matmuls, 1-2 m-chunks of W_ih_{l+1} @ hseq_l
(N=256 GEMMs over the previous block) are issued on the tensor queue.
They execute inside the per-step gate-math tail (otherwise PE-idle) and
keep the PE array busy enough that the HAM clock gate stays at 2.4 GHz.
Layer 0's input projection runs standalone up front (reads xT).

Weights are pre-scaled by SCL host-side; gates are computed at SCL*x
and descaled inside the tanh activations (free scale parameter).
Final FC done on-chip; host only transposes [96,16] -> [16,96] per core.
"""

import numpy as np
from contextlib import ExitStack

import concourse.bass as bass
import concourse.bacc as bacc
import concourse.mybir as mybir
import concourse.tile as tile
from concourse.bass_utils import run_bass_kernel_spmd

try:
    from ml_dtypes import bfloat16 as np_bf16
except ImportError:  # pragma: no cover
    import jax.numpy as jnp

    np_bf16 = jnp.bfloat16

B, T, IN, H, NLAYERS, PRED = 128, 512, 128, 1024, 3, 96
NCORES = 8
BC = B // NCORES  # 16
G = 3 * H  # 3072
NK = H // 128  # 8
NM = G // 128  # 24
BLK = 16  # recurrence steps per For_i iteration
BLKA = 4  # early sub-block of xi steps
CB = BLK * BC  # columns per block (256)
F32, BF16 = mybir.dt.float32, mybir.dt.bfloat16
SCL = 256.0  # weight pre-scale; descaled in the tanh activations
AF = mybir.ActivationFunctionType


def build(T_=T):
    nt = T_ * BC
    nblk = T_ // BLK
    ntile = nt // 512 if nt >= 512 else 1
    nc = bacc.Bacc("TRN2", target_bir_lowering=False, debug=False,
                   num_devices=NCORES)

    xT = nc.dram_tensor("xT", [128, nt], BF16, kind="ExternalInput")
    wihs = [
        nc.dram_tensor("wih0", [128, 1, G], BF16, kind="ExternalInput"),
        nc.dram_tensor("wih1", [128, NK, G], BF16, kind="ExternalInput"),
        nc.dram_tensor("wih2", [128, NK, G], BF16, kind="ExternalInput"),
    ]
    whhs = [nc.dram_tensor(f"whh{l}", [128, NK, G], BF16, kind="ExternalInput")
            for l in range(NLAYERS)]
    bizs = [nc.dram_tensor(f"biz{l}", [128, NM], F32, kind="ExternalInput")
            for l in range(NLAYERS)]
    bhns = [nc.dram_tensor(f"bhn{l}", [128, NK * BC], F32, kind="ExternalInput")
            for l in range(NLAYERS)]
    fcw = nc.dram_tensor("fcw", [128, NK, PRED], BF16, kind="ExternalInput")
    fcb = nc.dram_tensor("fcb", [PRED, 1], F32, kind="ExternalInput")
    out = nc.dram_tensor("out", [PRED, BC], F32, kind="ExternalOutput")

    with tile.TileContext(nc) as tc, ExitStack() as ctx:
        wpool = ctx.enter_context(tc.tile_pool(name="w", bufs=1))
        cpool = ctx.enter_context(tc.tile_pool(name="const", bufs=1))
        xpool = ctx.enter_context(tc.tile_pool(name="xT", bufs=1))
        rpool = ctx.enter_context(tc.tile_pool(name="rhs", bufs=2))
        epool = ctx.enter_context(tc.tile_pool(name="ev", bufs=3))
        xipool = ctx.enter_context(tc.tile_pool(name="xib", bufs=2))
        wnpool = ctx.enter_context(tc.tile_pool(name="win", bufs=2))
        tpool = ctx.enter_context(tc.tile_pool(name="tmp", bufs=2))
        spool = ctx.enter_context(tc.tile_pool(name="state", bufs=1))
        pspool = ctx.enter_context(tc.tile_pool(name="ps", bufs=2, space="PSUM"))
        psipool = ctx.enter_context(tc.tile_pool(name="psi", bufs=1, space="PSUM"))
        fcpool = ctx.enter_context(tc.tile_pool(name="psfc", bufs=1, space="PSUM"))
        pgpool = ctx.enter_context(tc.tile_pool(name="pg", bufs=1, space="PSUM"))
        dpool = ctx.enter_context(tc.tile_pool(name="dram", bufs=1, space="DRAM"))
        hqpool = ctx.enter_context(tc.tile_pool(name="hseq", bufs=2, space="DRAM"))

        # persistent state
        h32 = spool.tile([128, NK, BC], F32, tag="h32")
        h16 = spool.tile([128, NK, BC], BF16, tag="h16")

        xT_sb = xpool.tile([128, nt], BF16, tag="xT")
        nc.sync.dma_start(xT_sb[:], xT[:])
        fcw_sb = cpool.tile([128, NK, PRED], BF16, tag="fcw")
        nc.sync.dma_start(fcw_sb[:], fcw[:])
        fcb_sb = cpool.tile([PRED, 1], F32, tag="fcb")
        nc.sync.dma_start(fcb_sb[:], fcb[:])
        biz_sb = []
        bhn_sb = []
        for l in range(NLAYERS):
            t = cpool.tile([128, NM], F32, tag=f"biz{l}")
            nc.sync.dma_start(t[:], bizs[l][:])
            biz_sb.append(t)
            t = cpool.tile([128, NK * BC], F32, tag=f"bhn{l}")
            nc.sync.dma_start(t[:], bhns[l][:])
            bhn_sb.append(t)

        # ---------------- layer 0 input projection (standalone) ----------------
        wih_sb = wpool.tile([128, NK, G], BF16, tag="wih")
        nc.sync.dma_start(wih_sb[:, 0:1, :], wihs[0][:])
        xi = dpool.tile([128, NM, nt], BF16, tag="xi0")

        def l0_inproj(noff, ncols):
            for m in range(NM):
                ps = pspool.tile([128, 512], F32, tag="ps")
                nc.tensor.matmul(ps[:, 0:ncols], wih_sb[:, 0, m * 128:(m + 1) * 128],
                                 xT_sb[:, bass.ds(noff, ncols)], start=True, stop=True)
                ev = epool.tile([128, 512], BF16, tag="ev")
                nc.scalar.activation(ev[:, 0:ncols], ps[:, 0:ncols], AF.Identity,
                                     bias=biz_sb[0][:, m:m + 1], scale=SCL)
                nc.sync.dma_start(xi[:, m, bass.ds(noff, ncols)], ev[:, 0:ncols])

        if nt >= 512:
            with tc.For_i(0, ntile) as n:
                l0_inproj(n * 512, 512)
        else:
            l0_inproj(0, nt)

        # ---------------- layers ----------------
        for l in range(NLAYERS):
            last = l == NLAYERS - 1
            whh_sb = wpool.tile([128, NK, G], BF16, tag="whh")
            nc.sync.dma_start(whh_sb[:], whhs[l][:])
            if not last:
                wih_sb = wpool.tile([128, NK, G], BF16, tag="wih")
                nc.sync.dma_start(wih_sb[:], wihs[l + 1][:])
                hseq = hqpool.tile([128, NK, nt], BF16, tag="hseq")
                xi_next = dpool.tile([128, NM, nt], BF16, tag=f"xi{(l + 1) % 2}")

            nc.vector.memset(h32[:], 0.0)
            nc.vector.memset(h16[:], 0.0)

            def inproj_unit(m, src_off, dst_off):
                """One m-chunk of layer l+1's inproj over one CB-col block."""
                psI = psipool.tile([128, CB], F32, tag="psI")
                for k in range(NK):
                    nc.tensor.matmul(psI[:], wih_sb[:, k, m * 128:(m + 1) * 128],
                                     rhsI[:, k, :], start=(k == 0), stop=(k == NK - 1))
                evI = epool.tile([128, CB], BF16, tag="evI")
                nc.scalar.activation(evI[:], psI[:], AF.Identity,
                                     bias=biz_sb[l + 1][:, m:m + 1], scale=SCL)
                nc.sync.dma_start(xi_next[:, m, bass.ds(dst_off, CB)], evI[:])

            def step(dt, xiba, xibb, winb, inp_off):
                # gate matmuls: z (m 8..15), r (m 0..7), n (m 16..23).
                # k-half-major order: all (m, k 0..3) then (m, k 4..7), so the
                # NEXT step's k 0..3 matmuls depend only on the first half of
                # h16 — the second half of the gate tail hides under them.
                pgz = pgpool.tile([128, NK, BC], F32, tag="pgz")
                pgr = pgpool.tile([128, NK, BC], F32, tag="pgr")
                # n-gate PSUM in two tiles so the first half-tail's semaphore
                # fires after m 16..19 complete, ~1us before the burst ends
                pgnA = pgpool.tile([128, 4, BC], F32, tag="pgnA")
                pgnB = pgpool.tile([128, 4, BC], F32, tag="pgnB")
                for pg, mo, nm in ((pgz, 8, 8), (pgr, 0, 8),
                                   (pgnA, 16, 4), (pgnB, 20, 4)):
                    for m in range(nm):
                        for k in range(NK):
                            nc.tensor.matmul(
                                pg[:, m, :],
                                whh_sb[:, k, (m + mo) * 128:(m + mo + 1) * 128],
                                h16[:, k, :],
                                start=(k == 0), stop=(k == NK - 1))
                if dt < BLKA:
                    xs = xiba[:, :, dt * BC:(dt + 1) * BC]
                else:
                    xs = xibb[:, :, (dt - BLKA) * BC:(dt - BLKA + 1) * BC]
                # ---- z chain (overlaps r+n matmuls) ----
                zp = tpool.tile([128, 128], F32, tag="zp")
                nc.vector.tensor_add(zp[:], pgz[:], xs[:, 8:16, :])
                zt = tpool.tile([128, 128], F32, tag="zt")
                nc.scalar.activation(zt[:], zp[:], AF.Tanh, scale=0.5 / SCL)
                z = tpool.tile([128, 128], F32, tag="z")
                nc.vector.tensor_scalar(z[:], zt[:], 0.5, 0.5,
                                        mybir.AluOpType.mult,
                                        mybir.AluOpType.add)
                zc = tpool.tile([128, 128], F32, tag="zc")
                nc.vector.tensor_scalar(zc[:], zt[:], -0.5, 0.5,
                                        mybir.AluOpType.mult,
                                        mybir.AluOpType.add)
                zh = tpool.tile([128, 128], F32, tag="zh")
                nc.vector.tensor_mul(zh[:], z[:], h32[:])
                # ---- r chain (overlaps n matmuls) ----
                rp = tpool.tile([128, 128], F32, tag="rp")
                nc.vector.tensor_add(rp[:], pgr[:], xs[:, 0:8, :])
                rt = tpool.tile([128, 128], F32, tag="rt")
                nc.scalar.activation(rt[:], rp[:], AF.Tanh, scale=0.5 / SCL)
                r = tpool.tile([128, 128], F32, tag="r")
                nc.vector.tensor_scalar(r[:], rt[:], 0.5, 0.5,
                                        mybir.AluOpType.mult,
                                        mybir.AluOpType.add)
                # rb = r*b_hn + xs_n, precomputed off the critical tail
                rb1 = tpool.tile([128, 128], F32, tag="rb1")
                nc.vector.tensor_mul(rb1[:], r[:], bhn_sb[l][:])
                rb2 = tpool.tile([128, 128], F32, tag="rb2")
                nc.vector.tensor_add(rb2[:], rb1[:], xs[:, 16:24, :])
                # ---- n chain (the serial tail), two H-halves ----
                # matmul issue order is unchanged (k-inner); only the tail ops
                # are halved so h16[:, 0:4] lands early and the next step's
                # k 0..3 matmuls start while the second half still computes.
                p1h = {}
                for lo in (0, 64):
                    hs = slice(lo, lo + 64)
                    ks = slice(lo // 16, lo // 16 + 4)
                    t1 = tpool.tile([128, 64], F32, tag=f"t1{lo}")
                    nc.vector.tensor_mul(t1[:], r[:, hs],
                                         (pgnA if lo == 0 else pgnB)[:])
                    t2 = tpool.tile([128, 64], F32, tag=f"t2{lo}")
                    nc.vector.tensor_add(t2[:], t1[:], rb2[:, hs])
                    nt_ = tpool.tile([128, 64], F32, tag=f"nt{lo}")
                    nc.scalar.activation(nt_[:], t2[:], AF.Tanh, scale=1.0 / SCL)
                    p1 = tpool.tile([128, 64], F32, tag=f"p1{lo}")
                    nc.vector.tensor_mul(p1[:], zc[:, hs], nt_[:])
                    nc.vector.tensor_add(h16[:, ks, :], p1[:], zh[:, hs])
                    p1h[lo] = p1
                # off-critical writes after both h16 halves
                for lo in (0, 64):
                    hs = slice(lo, lo + 64)
                    ks = slice(lo // 16, lo // 16 + 4)
                    if winb is not None:
                        nc.vector.tensor_add(
                            winb[:, ks, dt * BC:(dt + 1) * BC], p1h[lo][:], zh[:, hs])
                    nc.vector.tensor_add(h32[:, ks, :], p1h[lo][:], zh[:, hs])
                # interleaved next-layer inproj: fills the per-step tail
                # (tensor-queue: after this step's MMs, before the next step's)
                if inp_off is not None:
                    sm = (3 * dt) // 2
                    cnt = (3 * (dt + 1)) // 2 - sm
                    for m in range(sm, sm + cnt):
                        inproj_unit(m, inp_off, inp_off)

            def block(rec_off, inp_off):
                """Recurrence block at column offset rec_off; optionally the
                interleaved next-layer inproj over block at inp_off."""
                xiba = xipool.tile([128, NM, BLKA * BC], BF16, tag="xiba")
                nc.sync.dma_start(xiba[:], xi[:, :, bass.ds(rec_off, BLKA * BC)])
                xibb = xipool.tile([128, NM, (BLK - BLKA) * BC], BF16, tag="xibb")
                nc.sync.dma_start(xibb[:], xi[:, :, bass.ds(rec_off + BLKA * BC,
                                                            (BLK - BLKA) * BC)])
                winb = None
                if not last:
                    winb = wnpool.tile([128, NK, CB], BF16, tag="win")
                for dt in range(BLK):
                    step(dt, xiba, xibb, winb, inp_off)
                if not last:
                    nc.sync.dma_start(hseq[:, :, bass.ds(rec_off, CB)], winb[:])

            # 2 blocks per For_i iteration: halves the ~10us per-iteration
            # cross-engine rendezvous bubble at hardware-loop boundaries
            if last:
                if nblk > 1:
                    with tc.For_i(0, nblk // 2) as blk:
                        block(blk * (2 * CB), None)
                        block(blk * (2 * CB) + CB, None)
                else:
                    block(0, None)
            else:
                rhsI = None
                block(0, None)  # peeled: no previous block to project yet
                if nblk > 1:
                    # blocks 1..nblk-1 with inproj lagging one block
                    with tc.For_i(0, (nblk - 2) // 2) as j:
                        rhsI = rpool.tile([128, NK, CB], BF16, tag="rhsI")
                        nc.sync.dma_start(rhsI[:],
                                          hseq[:, :, bass.ds(j * (2 * CB), CB)])
                        block(j * (2 * CB) + CB, j * (2 * CB))
                        rhsI = rpool.tile([128, NK, CB], BF16, tag="rhsI")
                        nc.sync.dma_start(rhsI[:],
                                          hseq[:, :, bass.ds(j * (2 * CB) + CB, CB)])
                        block(j * (2 * CB) + 2 * CB, j * (2 * CB) + CB)
                    # peeled final block (odd count): recurrence + inproj of prev
                    rhsI = rpool.tile([128, NK, CB], BF16, tag="rhsI")
                    nc.sync.dma_start(rhsI[:], hseq[:, :, bass.ds((nblk - 2) * CB, CB)])
                    block((nblk - 1) * CB, (nblk - 2) * CB)
                # trailing inproj for the final block of this layer
                rhsI = rpool.tile([128, NK, CB], BF16, tag="rhsI")
                fin = (nblk - 1) * CB
                nc.sync.dma_start(rhsI[:], hseq[:, :, bass.ds(fin, CB)])
                for m in range(NM):
                    inproj_unit(m, fin, fin)
            if not last:
                xi = xi_next

        # ---------------- final FC ----------------
        psfc = fcpool.tile([PRED, BC], F32, tag="psfc")
        for k in range(NK):
            nc.tensor.matmul(psfc[:], fcw_sb[:, k, :], h16[:, k, :],
                             start=(k == 0), stop=(k == NK - 1))
        ofc = epool.tile([PRED, BC], F32, tag="ofc")
        nc.scalar.activation(ofc[:], psfc[:], AF.Identity, bias=fcb_sb[:])
        nc.sync.dma_start(out[:], ofc[:])

    nc.compile()
    return nc


def prep_inputs(inputs, T_=T):
    """Host-side layout preprocessing. Returns per-core input maps."""
    x = np.asarray(inputs["x"], np.float32)

    def chunkT(w):  # [G_out, K*128] -> [128, K, G_out] (lhsT layout)
        w = np.asarray(w, np.float32)
        gout, kin = w.shape
        return np.ascontiguousarray(
            w.T.reshape(kin // 128, 128, gout).transpose(1, 0, 2)
        ).astype(np_bf16)

    def chunkT_scl(w):  # [G_out, K*128] -> [128, K, G_out], pre-scaled by SCL
        w = np.asarray(w, np.float32) * SCL
        gout, kin = w.shape
        return np.ascontiguousarray(
            w.T.reshape(kin // 128, 128, gout).transpose(1, 0, 2)
        ).astype(np_bf16)

    shared = {}
    for l in range(NLAYERS):
        wih = np.asarray(inputs[f"w_ih_{l}"], np.float32)
        shared[f"wih{l}"] = chunkT(wih)
        shared[f"whh{l}"] = chunkT_scl(inputs[f"w_hh_{l}"])
        b_ih = np.asarray(inputs[f"b_ih_{l}"], np.float32)
        b_hh = np.asarray(inputs[f"b_hh_{l}"], np.float32)
        comb = b_ih.copy()
        comb[:2 * H] += b_hh[:2 * H]
        shared[f"biz{l}"] = np.ascontiguousarray(comb.reshape(NM, 128).T) * SCL
        bhn = b_hh[2 * H:].reshape(NK, 128).T  # [128, NK]
        shared[f"bhn{l}"] = np.ascontiguousarray(np.repeat(bhn, BC, axis=1)) * SCL
    shared["fcw"] = chunkT(inputs["fc_w"])
    shared["fcb"] = np.asarray(inputs["fc_b"], np.float32).reshape(PRED, 1)

    in_maps = []
    for c in range(NCORES):
        xc = x[c * BC:(c + 1) * BC, :T_, :]  # [BC, T, IN]
        xTc = np.ascontiguousarray(
            xc.transpose(2, 1, 0).reshape(IN, T_ * BC)
        ).astype(np_bf16)
        m = dict(shared)
        m["xT"] = xTc
        in_maps.append(m)
    return in_maps


_NC_CACHE = {}


def kernel(**inputs):
    if "nc" not in _NC_CACHE:
        _NC_CACHE["nc"] = build()
    nc = _NC_CACHE["nc"]
    in_maps = prep_inputs(inputs)
    res = run_bass_kernel_spmd(nc, in_maps, list(range(NCORES)))
    outs = []
    for c in range(NCORES):
        o = np.asarray(res.results[c]["out"], np.float32)  # [PRED, BC]
        outs.append(o.T)  # [BC, PRED]
    return np.concatenate(outs, axis=0)  # [B, PRED]


if __name__ == "__main__":
    rng = np.random.default_rng(0)
    k = 1.0 / np.sqrt(H)
    ins = {"x": rng.standard_normal((B, T, IN), dtype=np.float32)}
    for l in range(NLAYERS):
        ind = IN if l == 0 else H
        ins[f"w_ih_{l}"] = rng.uniform(-k, k, (G, ind)).astype(np.float32)
        ins[f"w_hh_{l}"] = rng.uniform(-k, k, (G, H)).astype(np.float32)
        ins[f"b_ih_{l}"] = rng.uniform(-k, k, (G,)).astype(np.float32)
        ins[f"b_hh_{l}"] = rng.uniform(-k, k, (G,)).astype(np.float32)
    ins["fc_w"] = rng.uniform(-k, k, (PRED, H)).astype(np.float32)
    ins["fc_b"] = rng.uniform(-k, k, (PRED,)).astype(np.float32)
    print(kernel(**ins).shape)


# revision 32
# speedup vs baseline: 1.2620x; 1.0023x over previous
"""Trainium2 Bass kernel for a 3-layer GRU (B=128, T=512, IN=128, H=1024, PRED=96).

Strategy: data-parallel over batch across 8 NeuronCores (B_core=16).

Layer l's recurrence (phase B) runs 512 sequential steps; per step 192
bf16 matmuls (K=128, M=128, N=16) accumulate W_hh @ h_t into three PSUM
tiles (z, r, n gate groups, issued in that order) so the z/r sigmoid
chains on DVE/ACT overlap the remaining matmul stream; only the n-gate
tail (r*pg_n -> +rb -> tanh -> combine) is serial per step.

Layer l+1's input projection is absorbed into layer l's recurrence
loop: after each step's matmuls, 1-2 m-chunks of W_ih_{l+1} @ hseq_l
(N=256 GEMMs over the previous block) are issued on the tensor queue.
They execute inside the per-step gate-math tail (otherwise PE-idle) and
keep the PE array busy enough that the HAM clock gate stays at 2.4 GHz.
Layer 0's input projection runs standalone up front (reads xT).

Weights are pre-scaled by SCL host-side; gates are computed at SCL*x
and descaled inside the tanh activations (free scale parameter).
Final FC done on-chip; host only transposes [96,16] -> [16,96] per core.
"""

import numpy as np
from contextlib import ExitStack

import concourse.bass as bass
import concourse.bacc as bacc
import concourse.mybir as mybir
import concourse.tile as tile
from concourse.bass_utils import run_bass_kernel_spmd

try:
    from ml_dtypes import bfloat16 as np_bf16
except ImportError:  # pragma: no cover
    import jax.numpy as jnp

    np_bf16 = jnp.bfloat16

B, T, IN, H, NLAYERS, PRED = 128, 512, 128, 1024, 3, 96
NCORES = 8
BC = B // NCORES  # 16
G = 3 * H  # 3072
NK = H // 128  # 8
NM = G // 128  # 24
BLK = 16  # recurrence steps per For_i iteration
BLKA = 4  # early sub-block of xi steps
CB = BLK * BC  # columns per block (256)
F32, BF16 = mybir.dt.float32, mybir.dt.bfloat16
SCL = 256.0  # weight pre-scale; descaled in the tanh activations
AF = mybir.ActivationFunctionType


def build(T_=T):
    nt = T_ * BC
    nblk = T_ // BLK
    ntile = nt // 512 if nt >= 512 else 1
    nc = bacc.Bacc("TRN2", target_bir_lowering=False, debug=False,
                   num_devices=NCORES)

    xT = nc.dram_tensor("xT", [128, nt], BF16, kind="ExternalInput")
    wihs = [
        nc.dram_tensor("wih0", [128, 1, G], BF16, kind="ExternalInput"),
        nc.dram_tensor("wih1", [128, NK, G], BF16, kind="ExternalInput"),
        nc.dram_tensor("wih2", [128, NK, G], BF16, kind="ExternalInput"),
    ]
    whhs = [nc.dram_tensor(f"whh{l}", [128, NK, G], BF16, kind="ExternalInput")
            for l in range(NLAYERS)]
    bizs = [nc.dram_tensor(f"biz{l}", [128, NM], F32, kind="ExternalInput")
            for l in range(NLAYERS)]
    bhns = [nc.dram_tensor(f"bhn{l}", [128, NK * BC], F32, kind="ExternalInput")
            for l in range(NLAYERS)]
    fcw = nc.dram_tensor("fcw", [128, NK, PRED], BF16, kind="ExternalInput")
    fcb = nc.dram_tensor("fcb", [PRED, 1], F32, kind="ExternalInput")
    out = nc.dram_tensor("out", [PRED, BC], F32, kind="ExternalOutput")

    with tile.TileContext(nc) as tc, ExitStack() as ctx:
        wpool = ctx.enter_context(tc.tile_pool(name="w", bufs=1))
        cpool = ctx.enter_context(tc.tile_pool(name="const", bufs=1))
        xpool = ctx.enter_context(tc.tile_pool(name="xT", bufs=1))
        rpool = ctx.enter_context(tc.tile_pool(name="rhs", bufs=2))
        epool = ctx.enter_context(tc.tile_pool(name="ev", bufs=3))
        xipool = ctx.enter_context(tc.tile_pool(name="xib", bufs=2))
        wnpool = ctx.enter_context(tc.tile_pool(name="win", bufs=2))
        tpool = ctx.enter_context(tc.tile_pool(name="tmp", bufs=2))
        spool = ctx.enter_context(tc.tile_pool(name="state", bufs=1))
        pspool = ctx.enter_context(tc.tile_pool(name="ps", bufs=2, space="PSUM"))
        psipool = ctx.enter_context(tc.tile_pool(name="psi", bufs=1, space="PSUM"))
        fcpool = ctx.enter_context(tc.tile_pool(name="psfc", bufs=1, space="PSUM"))
        pgpool = ctx.enter_context(tc.tile_pool(name="pg", bufs=1, space="PSUM"))
        dpool = ctx.enter_context(tc.tile_pool(name="dram", bufs=1, space="DRAM"))
        hqpool = ctx.enter_context(tc.tile_pool(name="hseq", bufs=2, space="DRAM"))

        # persistent state
        h32 = spool.tile([128, NK, BC], F32, tag="h32")
        h16 = spool.tile([128, NK, BC], BF16, tag="h16")

        xT_sb = xpool.tile([128, nt], BF16, tag="xT")
        nc.sync.dma_start(xT_sb[:], xT[:])
        fcw_sb = cpool.tile([128, NK, PRED], BF16, tag="fcw")
        nc.sync.dma_start(fcw_sb[:], fcw[:])
        fcb_sb = cpool.tile([PRED, 1], F32, tag="fcb")
        nc.sync.dma_start(fcb_sb[:], fcb[:])
        biz_sb = []
        bhn_sb = []
        for l in range(NLAYERS):
            t = cpool.tile([128, NM], F32, tag=f"biz{l}")
            nc.sync.dma_start(t[:], bizs[l][:])
            biz_sb.append(t)
            t = cpool.tile([128, NK * BC], F32, tag=f"bhn{l}")
            nc.sync.dma_start(t[:], bhns[l][:])
            bhn_sb.append(t)

        # ---------------- layer 0 input projection (standalone) ----------------
        wih_sb = wpool.tile([128, NK, G], BF16, tag="wih")
        nc.sync.dma_start(wih_sb[:, 0:1, :], wihs[0][:])
        xi = dpool.tile([128, NM, nt], BF16, tag="xi0")

        def l0_inproj(noff, ncols):
            for m in range(NM):
                ps = pspool.tile([128, 512], F32, tag="ps")
                nc.tensor.matmul(ps[:, 0:ncols], wih_sb[:, 0, m * 128:(m + 1) * 128],
                                 xT_sb[:, bass.ds(noff, ncols)], start=True, stop=True)
                ev = epool.tile([128, 512], BF16, tag="ev")
                nc.scalar.activation(ev[:, 0:ncols], ps[:, 0:ncols], AF.Identity,
                                     bias=biz_sb[0][:, m:m + 1], scale=SCL)
                nc.sync.dma_start(xi[:, m, bass.ds(noff, ncols)], ev[:, 0:ncols])

        if nt >= 1024:
            with tc.For_i(0, ntile // 2) as n:
                l0_inproj(n * 1024, 512)
                l0_inproj(n * 1024 + 512, 512)
        elif nt >= 512:
            with tc.For_i(0, ntile) as n:
                l0_inproj(n * 512, 512)
        else:
            l0_inproj(0, nt)

        # ---------------- layers ----------------
        for l in range(NLAYERS):
            last = l == NLAYERS - 1
            whh_sb = wpool.tile([128, NK, G], BF16, tag="whh")
            nc.sync.dma_start(whh_sb[:], whhs[l][:])
            if not last:
                wih_sb = wpool.tile([128, NK, G], BF16, tag="wih")
                nc.sync.dma_start(wih_sb[:], wihs[l + 1][:])
                hseq = hqpool.tile([128, NK, nt], BF16, tag="hseq")
                xi_next = dpool.tile([128, NM, nt], BF16, tag=f"xi{(l + 1) % 2}")

            nc.vector.memset(h32[:], 0.0)
            nc.vector.memset(h16[:], 0.0)

            def inproj_unit(m, src_off, dst_off):
                """One m-chunk of layer l+1's inproj over one CB-col block."""
                psI = psipool.tile([128, CB], F32, tag="psI")
                for k in range(NK):
                    nc.tensor.matmul(psI[:], wih_sb[:, k, m * 128:(m + 1) * 128],
                                     rhsI[:, k, :], start=(k == 0), stop=(k == NK - 1))
                evI = epool.tile([128, CB], BF16, tag="evI")
                nc.scalar.activation(evI[:], psI[:], AF.Identity,
                                     bias=biz_sb[l + 1][:, m:m + 1], scale=SCL)
                nc.sync.dma_start(xi_next[:, m, bass.ds(dst_off, CB)], evI[:])

            def step(dt, xiba, xibb, winb, inp_off):
                # gate matmuls: z (m 8..15), r (m 0..7), n (m 16..23).
                # k-half-major order: all (m, k 0..3) then (m, k 4..7), so the
                # NEXT step's k 0..3 matmuls depend only on the first half of
                # h16 — the second half of the gate tail hides under them.
                pgz = pgpool.tile([128, NK, BC], F32, tag="pgz")
                pgr = pgpool.tile([128, NK, BC], F32, tag="pgr")
                # n-gate PSUM in two tiles so the first half-tail's semaphore
                # fires after m 16..19 complete, ~1us before the burst ends
                pgnA = pgpool.tile([128, 4, BC], F32, tag="pgnA")
                pgnB = pgpool.tile([128, 4, BC], F32, tag="pgnB")
                for pg, mo, nm in ((pgz, 8, 8), (pgr, 0, 8),
                                   (pgnA, 16, 4), (pgnB, 20, 4)):
                    for m in range(nm):
                        for k in range(NK):
                            nc.tensor.matmul(
                                pg[:, m, :],
                                whh_sb[:, k, (m + mo) * 128:(m + mo + 1) * 128],
                                h16[:, k, :],
                                start=(k == 0), stop=(k == NK - 1))
                if dt < BLKA:
                    xs = xiba[:, :, dt * BC:(dt + 1) * BC]
                else:
                    xs = xibb[:, :, (dt - BLKA) * BC:(dt - BLKA + 1) * BC]
                # ---- z chain (overlaps r+n matmuls) ----
                zp = tpool.tile([128, 128], F32, tag="zp")
                nc.vector.tensor_add(zp[:], pgz[:], xs[:, 8:16, :])
                zt = tpool.tile([128, 128], F32, tag="zt")
                nc.scalar.activation(zt[:], zp[:], AF.Tanh, scale=0.5 / SCL)
                z = tpool.tile([128, 128], F32, tag="z")
                nc.vector.tensor_scalar(z[:], zt[:], 0.5, 0.5,
                                        mybir.AluOpType.mult,
                                        mybir.AluOpType.add)
                zc = tpool.tile([128, 128], F32, tag="zc")
                nc.vector.tensor_scalar(zc[:], zt[:], -0.5, 0.5,
                                        mybir.AluOpType.mult,
                                        mybir.AluOpType.add)
                zh = tpool.tile([128, 128], F32, tag="zh")
                nc.vector.tensor_mul(zh[:], z[:], h32[:])
                # ---- r chain (overlaps n matmuls) ----
                rp = tpool.tile([128, 128], F32, tag="rp")
                nc.vector.tensor_add(rp[:], pgr[:], xs[:, 0:8, :])
                rt = tpool.tile([128, 128], F32, tag="rt")
                nc.scalar.activation(rt[:], rp[:], AF.Tanh, scale=0.5 / SCL)
                r = tpool.tile([128, 128], F32, tag="r")
                nc.vector.tensor_scalar(r[:], rt[:], 0.5, 0.5,
                                        mybir.AluOpType.mult,
                                        mybir.AluOpType.add)
                # rb = r*b_hn + xs_n, precomputed off the critical tail
                rb1 = tpool.tile([128, 128], F32, tag="rb1")
                nc.vector.tensor_mul(rb1[:], r[:], bhn_sb[l][:])
                rb2 = tpool.tile([128, 128], F32, tag="rb2")
                nc.vector.tensor_add(rb2[:], rb1[:], xs[:, 16:24, :])
                # ---- n chain (the serial tail), two H-halves ----
                # matmul issue order is unchanged (k-inner); only the tail ops
                # are halved so h16[:, 0:4] lands early and the next step's
                # k 0..3 matmuls start while the second half still computes.
                p1h = {}
                for lo in (0, 64):
                    hs = slice(lo, lo + 64)
                    ks = slice(lo // 16, lo // 16 + 4)
                    t1 = tpool.tile([128, 64], F32, tag=f"t1{lo}")
                    nc.vector.tensor_mul(t1[:], r[:, hs],
                                         (pgnA if lo == 0 else pgnB)[:])
                    t2 = tpool.tile([128, 64], F32, tag=f"t2{lo}")
                    nc.vector.tensor_add(t2[:], t1[:], rb2[:, hs])
                    nt_ = tpool.tile([128, 64], F32, tag=f"nt{lo}")
                    nc.scalar.activation(nt_[:], t2[:], AF.Tanh, scale=1.0 / SCL)
                    p1 = tpool.tile([128, 64], F32, tag=f"p1{lo}")
                    nc.vector.tensor_mul(p1[:], zc[:, hs], nt_[:])
                    nc.vector.tensor_add(h16[:, ks, :], p1[:], zh[:, hs])
                    p1h[lo] = p1
                # off-critical writes after both h16 halves
                for lo in (0, 64):
                    hs = slice(lo, lo + 64)
                    ks = slice(lo // 16, lo // 16 + 4)
                    if winb is not None:
                        nc.vector.tensor_add(
                            winb[:, ks, dt * BC:(dt + 1) * BC], p1h[lo][:], zh[:, hs])
                    nc.vector.tensor_add(h32[:, ks, :], p1h[lo][:], zh[:, hs])
                # interleaved next-layer inproj: fills the per-step tail
                # (tensor-queue: after this step's MMs, before the next step's)
                if inp_off is not None:
                    sm = (3 * dt) // 2
                    cnt = (3 * (dt + 1)) // 2 - sm
                    for m in range(sm, sm + cnt):
                        inproj_unit(m, inp_off, inp_off)

            def block(rec_off, inp_off):
                """Recurrence block at column offset rec_off; optionally the
                interleaved next-layer inproj over block at inp_off."""
                xiba = xipool.tile([128, NM, BLKA * BC], BF16, tag="xiba")
                nc.sync.dma_start(xiba[:], xi[:, :, bass.ds(rec_off, BLKA * BC)])
                xibb = xipool.tile([128, NM, (BLK - BLKA) * BC], BF16, tag="xibb")
                nc.sync.dma_start(xibb[:], xi[:, :, bass.ds(rec_off + BLKA * BC,
                                                            (BLK - BLKA) * BC)])
                winb = None
                if not last:
                    winb = wnpool.tile([128, NK, CB], BF16, tag="win")
                for dt in range(BLK):
                    step(dt, xiba, xibb, winb, inp_off)
                if not last:
                    nc.sync.dma_start(hseq[:, :, bass.ds(rec_off, CB)], winb[:])

            # 2 blocks per For_i iteration: halves the ~10us per-iteration
            # cross-engine rendezvous bubble at hardware-loop boundaries
            if last:
                if nblk >= 4:
                    with tc.For_i(0, nblk // 4) as blk:
                        for u in range(4):
                            block(blk * (4 * CB) + u * CB, None)
                elif nblk > 1:
                    with tc.For_i(0, nblk // 2) as blk:
                        block(blk * (2 * CB), None)
                        block(blk * (2 * CB) + CB, None)
                else:
                    block(0, None)
            else:
                rhsI = None
                block(0, None)  # peeled: no previous block to project yet
                if nblk > 1:
                    # blocks 1..nblk-2 in a 3x-unrolled loop, inproj lagging one
                    assert (nblk - 2) % 3 == 0
                    with tc.For_i(0, (nblk - 2) // 3) as j:
                        for u in range(3):
                            rhsI = rpool.tile([128, NK, CB], BF16, tag="rhsI")
                            nc.sync.dma_start(
                                rhsI[:],
                                hseq[:, :, bass.ds(j * (3 * CB) + u * CB, CB)])
                            block(j * (3 * CB) + (u + 1) * CB,
                                  j * (3 * CB) + u * CB)
                    # peeled final block: recurrence + inproj of prev
                    rhsI = rpool.tile([128, NK, CB], BF16, tag="rhsI")
                    nc.sync.dma_start(rhsI[:], hseq[:, :, bass.ds((nblk - 2) * CB, CB)])
                    block((nblk - 1) * CB, (nblk - 2) * CB)
                # trailing inproj for the final block of this layer
                rhsI = rpool.tile([128, NK, CB], BF16, tag="rhsI")
                fin = (nblk - 1) * CB
                nc.sync.dma_start(rhsI[:], hseq[:, :, bass.ds(fin, CB)])
                for m in range(NM):
                    inproj_unit(m, fin, fin)
            if not last:
                xi = xi_next

        # ---------------- final FC ----------------
        psfc = fcpool.tile([PRED, BC], F32, tag="psfc")
        for k in range(NK):
            nc.tensor.matmul(psfc[:], fcw_sb[:, k, :], h16[:, k, :],
                             start=(k == 0), stop=(k == NK - 1))
        ofc = epool.tile([PRED, BC], F32, tag="ofc")
        nc.scalar.activation(ofc[:], psfc[:], AF.Identity, bias=fcb_sb[:])
        nc.sync.dma_start(out[:], ofc[:])

    nc.compile()
    return nc


def prep_inputs(inputs, T_=T):
    """Host-side layout preprocessing. Returns per-core input maps."""
    x = np.asarray(inputs["x"], np.float32)

    def chunkT(w):  # [G_out, K*128] -> [128, K, G_out] (lhsT layout)
        w = np.asarray(w, np.float32)
        gout, kin = w.shape
        return np.ascontiguousarray(
            w.T.reshape(kin // 128, 128, gout).transpose(1, 0, 2)
        ).astype(np_bf16)

    def chunkT_scl(w):  # [G_out, K*128] -> [128, K, G_out], pre-scaled by SCL
        w = np.asarray(w, np.float32) * SCL
        gout, kin = w.shape
        return np.ascontiguousarray(
            w.T.reshape(kin // 128, 128, gout).transpose(1, 0, 2)
        ).astype(np_bf16)

    shared = {}
    for l in range(NLAYERS):
        wih = np.asarray(inputs[f"w_ih_{l}"], np.float32)
        shared[f"wih{l}"] = chunkT(wih)
        shared[f"whh{l}"] = chunkT_scl(inputs[f"w_hh_{l}"])
        b_ih = np.asarray(inputs[f"b_ih_{l}"], np.float32)
        b_hh = np.asarray(inputs[f"b_hh_{l}"], np.float32)
        comb = b_ih.copy()
        comb[:2 * H] += b_hh[:2 * H]
        shared[f"biz{l}"] = np.ascontiguousarray(comb.reshape(NM, 128).T) * SCL
        bhn = b_hh[2 * H:].reshape(NK, 128).T  # [128, NK]
        shared[f"bhn{l}"] = np.ascontiguousarray(np.repeat(bhn, BC, axis=1)) * SCL
    shared["fcw"] = chunkT(inputs["fc_w"])
    shared["fcb"] = np.asarray(inputs["fc_b"], np.float32).reshape(PRED, 1)

    in_maps = []
    for c in range(NCORES):
        xc = x[c * BC:(c + 1) * BC, :T_, :]  # [BC, T, IN]
        xTc = np.ascontiguousarray(
            xc.transpose(2, 1, 0).reshape(IN, T_ * BC)
        ).astype(np_bf16)
        m = dict(shared)
        m["xT"] = xTc
        in_maps.append(m)
    return in_maps


_NC_CACHE = {}


def kernel(**inputs):
    if "nc" not in _NC_CACHE:
        _NC_CACHE["nc"] = build()
    nc = _NC_CACHE["nc"]
    in_maps = prep_inputs(inputs)
    res = run_bass_kernel_spmd(nc, in_maps, list(range(NCORES)))
    outs = []
    for c in range(NCORES):
        o = np.asarray(res.results[c]["out"], np.float32)  # [PRED, BC]
        outs.append(o.T)  # [BC, PRED]
    return np.concatenate(outs, axis=0)  # [B, PRED]


if __name__ == "__main__":
    rng = np.random.default_rng(0)
    k = 1.0 / np.sqrt(H)
    ins = {"x": rng.standard_normal((B, T, IN), dtype=np.float32)}
    for l in range(NLAYERS):
        ind = IN if l == 0 else H
        ins[f"w_ih_{l}"] = rng.uniform(-k, k, (G, ind)).astype(np.float32)
        ins[f"w_hh_{l}"] = rng.uniform(-k, k, (G, H)).astype(np.float32)
        ins[f"b_ih_{l}"] = rng.uniform(-k, k, (G,)).astype(np.float32)
        ins[f"b_hh_{l}"] = rng.uniform(-k, k, (G,)).astype(np.float32)
    ins["fc_w"] = rng.uniform(-k, k, (PRED, H)).astype(np.float32)
    ins["fc_b"] = rng.uniform(-k, k, (PRED,)).astype(np.float32)
    print(kernel(**ins).shape)
